# revision 1
# baseline (speedup 1.0000x reference)
"""Causal self-attention with RoPE (B=2, T=2048, C=2048, 16 heads) on 8 TRN2
NeuronCores.

Sharding: data-parallel over batch x tensor-parallel over heads.
Core c handles batch c//4 and heads 4*(c%4) .. 4*(c%4)+4. Each core computes
its heads' q/k/v projections, RoPE, causal attention, and a partial output
projection over its heads' channels; the host sums the 4 partial projections
per batch (the tensor-parallel reduce) and stacks the batches.

Per-core program (fp16 operands, fp32 accumulation):
  A1: qT/kT[hd, T] = (w_qk chunk).T @ xT accumulated over C chunks in PSUM;
      RoPE applied straight out of PSUM (DVE products, GPSIMD combines). q/k
      weight columns are host-permuted even-first so rotation pairs sit in
      partition halves.
  A2: v[T, hd*4] = (xT chunk).T @ w_v.
  B:  per (head, 512-wide q tile): for each causal 128-chunk of k:
      scoresT = kT_chunk.T @ qT_tile -> PSUM; att = exp(scale*scoresT) (ACT);
      diagonal chunks masked post-exp (DVE); yT += v_chunk.T @ att and
      denom += ones.T @ att accumulate in PSUM; then yT *= 1/denom.
  C:  outT[C, T] partial = (w_proj chunk).T @ yT.
"""
import os
import numpy as np
from contextlib import ExitStack

os.environ.setdefault("JAX_COMPILATION_CACHE_DIR", "/tmp/jax_comp_cache")

import concourse.bass as bass
import concourse.tile as tile
from concourse import bacc, mybir
from concourse import bass_isa
from concourse.bass_utils import run_bass_kernel_spmd
from concourse._compat import axon_active

FP16 = mybir.dt.float16
FP32 = mybir.dt.float32

B, T, C, NH = 2, 2048, 2048, 16
HD = C // NH
N_CORES = 8
GROUPS = N_CORES // B
HPC = NH // GROUPS
QN = 512
AN = 1024


def _build_nc(T=2048, C=2048, HPC=4, n_cores=8, qn=512, reps=1, an=AN):
    """HPC = heads per core; head_dim fixed 128. qn = moving free-dim tile."""
    HD = 128
    CH = C // 128          # contraction chunks
    QK_COLS = HPC * HD     # q cols (= k cols) per core
    V_COLS = HPC * HD
    NQT = T // qn          # q tiles in attention
    NKC = T // 128         # k chunks
    scale = 1.0 / np.sqrt(np.float32(HD))

    nc = bacc.Bacc("TRN2", target_bir_lowering=False, debug=False,
                   num_devices=n_cores)
    xT_ap = nc.dram_tensor("xT", (C, T), FP16, kind="ExternalInput").ap()
    wqk_ap = nc.dram_tensor("wqk", (C, 2 * QK_COLS), FP16, kind="ExternalInput").ap()
    wv_ap = nc.dram_tensor("wv", (C, V_COLS), FP16, kind="ExternalInput").ap()
    wp_ap = nc.dram_tensor("wp", (HPC * HD, C), FP16, kind="ExternalInput").ap()
    cos_ap = nc.dram_tensor("cosT", (64, T), FP32, kind="ExternalInput").ap()
    sin_ap = nc.dram_tensor("sinT", (64, T), FP32, kind="ExternalInput").ap()
    mask_ap = nc.dram_tensor("masks", (128, 4 * qn), FP16, kind="ExternalInput").ap()
    out_ap = nc.dram_tensor("outT", (C, T), FP16, kind="ExternalOutput").ap()

    with tile.TileContext(nc) as tc:
      for rep in range(reps):
        R = f"r{rep}_"
        with ExitStack() as top:
            xt_pool = top.enter_context(tc.tile_pool(name=R+"xt", bufs=1))
            qk_pool = top.enter_context(tc.tile_pool(name=R+"qk", bufs=1))

            xt = xt_pool.tile([128, CH, T], FP16)
            qk_sb = qk_pool.tile([128, 2 * HPC, T], FP16)   # [hd, col, T]; cols 0..HPC-1 q, HPC.. k

            # ---- Phase A1: q/k projection + RoPE ----
            with ExitStack() as sA1:
                wqk_pool = sA1.enter_context(tc.tile_pool(name=R+"wqk", bufs=1))
                tab_pool = sA1.enter_context(tc.tile_pool(name=R+"tab", bufs=1))
                tmp_pool = sA1.enter_context(tc.tile_pool(name=R+"tmp", bufs=2))
                psA_pool = sA1.enter_context(tc.tile_pool(name=R+"psA", bufs=4, space="PSUM"))

                wqk = wqk_pool.tile([128, CH, 2 * QK_COLS], FP16)
                cos_t = tab_pool.tile([64, T], FP32)
                sin_t = tab_pool.tile([64, T], FP32)
                warm = tab_pool.tile([1, 1], FP32)
                nc.vector.memset(warm[:], 0.0)
                warm2 = tab_pool.tile([1, 1], FP32)
                nc.scalar.activation(warm2[:], warm[:],
                                     mybir.ActivationFunctionType.Exp)
                for ch in range(CH):
                    nc.sync.dma_start(wqk[:, ch, :],
                                      wqk_ap.rearrange("(ch p) n -> p ch n", p=128)[:, ch, :])
                    nc.sync.dma_start(xt[:, ch, :],
                                      xT_ap.rearrange("(ch p) t -> p ch t", p=128)[:, ch, :])
                    if ch == 1:
                        nc.sync.dma_start(cos_t[:], cos_ap[:])
                        nc.sync.dma_start(sin_t[:], sin_ap[:])

                def rope_emit(col, tq, ps):
                    cs = cos_t[:, tq * qn:(tq + 1) * qn]
                    sn = sin_t[:, tq * qn:(tq + 1) * qn]
                    dst = qk_sb[:, col, tq * qn:(tq + 1) * qn]
                    t1 = tmp_pool.tile([64, qn], FP32, tag="t1", name=f"{R}t1_{col}_{tq}")
                    t2 = tmp_pool.tile([64, qn], FP32, tag="t2", name=f"{R}t2_{col}_{tq}")
                    nc.vector.tensor_mul(t1[:], ps[0:64, :], cs)
                    nc.vector.tensor_mul(t2[:], ps[64:128, :], sn)
                    nc.gpsimd.tensor_sub(dst[0:64, :], t1[:], t2[:])
                    t3 = tmp_pool.tile([64, qn], FP32, tag="t3", name=f"{R}t3_{col}_{tq}")
                    t4 = tmp_pool.tile([64, qn], FP32, tag="t4", name=f"{R}t4_{col}_{tq}")
                    nc.vector.tensor_mul(t3[:], ps[0:64, :], sn)
                    nc.vector.tensor_mul(t4[:], ps[64:128, :], cs)
                    nc.gpsimd.tensor_add(dst[64:128, :], t3[:], t4[:])

                tiles = [(col, tq) for col in range(2 * HPC)
                         for tq in range(T // qn)]
                grp = tiles[:4]
                grp_ps = []
                for (col, tq) in grp:
                    gps = psA_pool.tile([128, qn], FP32, tag="psA",
                                        name=f"{R}psA_{col}_{tq}")
                    grp_ps.append(gps)
                for ch in range(CH):
                    for gi, (col, tq) in enumerate(grp):
                        nc.tensor.matmul(
                            grp_ps[gi][:], wqk[:, ch, col * HD:(col + 1) * HD],
                            xt[:, ch, tq * qn:(tq + 1) * qn],
                            start=(ch == 0), stop=(ch == CH - 1))
                for gi, (col, tq) in enumerate(grp):
                    rope_emit(col, tq, grp_ps[gi])
                for (col, tq) in tiles[4:]:
                    ps = psA_pool.tile([128, qn], FP32, tag="psA",
                                       name=f"{R}psA_{col}_{tq}")
                    for ch in range(CH):
                        nc.tensor.matmul(
                            ps[:], wqk[:, ch, col * HD:(col + 1) * HD],
                            xt[:, ch, tq * qn:(tq + 1) * qn],
                            start=(ch == 0), stop=(ch == CH - 1))
                    rope_emit(col, tq, ps)

            # ---- Phase A2: v projection ----
            with ExitStack() as sV:
                v_pool = sV.enter_context(tc.tile_pool(name=R+"v", bufs=1))
                v_sb = v_pool.tile([128, NKC, V_COLS], FP16)   # [t_lo, t_chunk, vcol]
                with ExitStack() as sA2:
                    wv_pool = sA2.enter_context(tc.tile_pool(name=R+"wv", bufs=1))
                    psV_pool = sA2.enter_context(tc.tile_pool(name=R+"psV", bufs=3, space="PSUM"))
                    wv = wv_pool.tile([128, CH, V_COLS], FP16)
                    for ch in range(CH):
                        nc.sync.dma_start(wv[:, ch, :],
                                          wv_ap.rearrange("(ch p) n -> p ch n", p=128)[:, ch, :])
                    for ts in range(NKC):
                        psv = psV_pool.tile([128, V_COLS], FP32, tag="psV",
                                            name=f"{R}psV_{ts}")
                        for ch in range(CH):
                            nc.tensor.matmul(
                                psv[:], xt[:, ch, ts * 128:(ts + 1) * 128],
                                wv[:, ch, :],
                                start=(ch == 0), stop=(ch == CH - 1))
                        nc.scalar.copy(v_sb[:, ts, :], psv[:])

                # ---- Phase B: attention ----
                with ExitStack() as sY:
                    yt_pool = sY.enter_context(tc.tile_pool(name=R+"yt", bufs=1))
                    yt_sb = yt_pool.tile([128, HPC, T], FP16)
                    with ExitStack() as sB:
                        msk_pool = sB.enter_context(tc.tile_pool(name=R+"msk", bufs=1))
                        one_pool = sB.enter_context(tc.tile_pool(name=R+"one", bufs=1))
                        att_pool = sB.enter_context(tc.tile_pool(name=R+"att", bufs=6))
                        rec_pool = sB.enter_context(tc.tile_pool(name=R+"rec", bufs=2))
                        psS_pool = sB.enter_context(tc.tile_pool(name=R+"psS", bufs=3, space="PSUM"))
                        psY_pool = sB.enter_context(tc.tile_pool(name=R+"psY", bufs=2, space="PSUM"))
                        psD_pool = sB.enter_context(tc.tile_pool(name=R+"psD", bufs=1, space="PSUM"))

                        wp_pool = sB.enter_context(tc.tile_pool(name=R+"wp", bufs=1))
                        o_pool = sB.enter_context(tc.tile_pool(name=R+"o", bufs=4))
                        psO_pool = sB.enter_context(tc.tile_pool(name=R+"psO", bufs=2, space="PSUM"))
                        msk = msk_pool.tile([128, 4, qn], FP16)
                        nc.sync.dma_start(msk[:], mask_ap.rearrange("p (j n) -> p j n", n=qn))
                        ones_t = one_pool.tile([128, 1], FP16)
                        nc.vector.memset(ones_t[:], 1.0)
                        wp = wp_pool.tile([128, HPC, C], FP16)
                        nc.sync.dma_start(wp[:], wp_ap.rearrange("(hh p) c -> p hh c", p=128))

                        diag_per_qt = qn // 128
                        NCT = C // 128
                        for qt in range(NQT):
                            for h in range(HPC):
                                nch = diag_per_qt * (qt + 1)
                                psY = psY_pool.tile([128, qn], FP32, tag="psY",
                                                    name=f"{R}psY_{h}_{qt}")
                                psD = psD_pool.tile([1, qn], FP32, tag="psD",
                                                    name=f"{R}psD_{h}_{qt}")
                                for kc in range(nch):
                                    psS = psS_pool.tile([128, qn], FP32, tag="psS",
                                                        name=f"{R}psS_{h}_{qt}_{kc}")
                                    nc.tensor.matmul(
                                        psS[:],
                                        qk_sb[:, HPC + h, kc * 128:(kc + 1) * 128],
                                        qk_sb[:, h, qt * qn:(qt + 1) * qn],
                                        start=True, stop=True)
                                    att = att_pool.tile([128, qn], FP16, tag="att",
                                                        name=f"{R}att_{h}_{qt}_{kc}")
                                    nc.scalar.activation(
                                        att[:], psS[:],
                                        mybir.ActivationFunctionType.Exp,
                                        scale=float(scale))
                                    j = kc - diag_per_qt * qt
                                    if j >= 0:
                                        nc.vector.tensor_mul(att[:], att[:], msk[:, j, :])
                                    nc.tensor.matmul(
                                        psY[:], v_sb[:, kc, h * HD:(h + 1) * HD], att[:],
                                        start=(kc == 0), stop=(kc == nch - 1))
                                    nc.tensor.matmul(
                                        psD[:], ones_t[:], att[:],
                                        start=(kc == 0), stop=(kc == nch - 1))
                                rec = rec_pool.tile([1, qn], FP32, tag="rec",
                                                    name=f"{R}rec_{h}_{qt}")
                                nc.vector.reciprocal(rec[:], psD[:])
                                recb = rec_pool.tile([128, qn], FP32, tag="recb",
                                                     name=f"{R}recb_{h}_{qt}")
                                nc.gpsimd.partition_broadcast(recb[:], rec[:])
                                dst = yt_sb[:, h, qt * qn:(qt + 1) * qn]
                                nc.vector.tensor_mul(dst, psY[:], recb[:])
                            # projection for this q-tile (all heads ready)
                            for ct in range(NCT):
                                pso = psO_pool.tile([128, 512], FP32, tag="psO",
                                                    name=f"{R}psO_{ct}_{qt}")
                                for hh in range(HPC):
                                    nc.tensor.matmul(
                                        pso[:], wp[:, hh, ct * 128:(ct + 1) * 128],
                                        yt_sb[:, hh, qt * qn:(qt + 1) * qn],
                                        start=(hh == 0), stop=(hh == HPC - 1))
                                o_t = o_pool.tile([128, 512], FP16, tag="o",
                                                  name=f"{R}o_{ct}_{qt}")
                                nc.vector.tensor_copy(o_t[:], pso[:])
                                nc.sync.dma_start(
                                    out_ap[ct * 128:(ct + 1) * 128, qt * qn:(qt + 1) * qn],
                                    o_t[:])
    nc.compile()
    return nc


_CACHE = {}


def _rope_tables_np(t_len, hd):
    inv_freq = 1.0 / (10000.0 ** (np.arange(0, hd, 2, dtype=np.float32) / hd))
    t = np.arange(t_len, dtype=np.float32)
    freqs = np.outer(t, inv_freq)
    emb = np.concatenate([freqs, freqs], axis=-1)
    return np.cos(emb)[:, ::2].astype(np.float32), np.sin(emb)[:, ::2].astype(np.float32)


def _static_arrays():
    if "static" not in _CACHE:
        cos_, sin_ = _rope_tables_np(T, HD)
        cosT = np.ascontiguousarray(cos_.T)
        sinT = np.ascontiguousarray(sin_.T)
        perm = np.concatenate([np.arange(0, HD, 2), np.arange(1, HD, 2)])
        p = np.arange(128)[:, None]
        f = np.arange(QN)[None, :]
        masks = np.concatenate(
            [(p <= (f - 128 * j)).astype(np.float16) for j in range(QN // 128)],
            axis=1)
        _CACHE["static"] = (cosT, sinT, perm, masks)
    return _CACHE["static"]


def _host_prep(x, w_qkv, w_proj):
    cosT, sinT, perm, masks = _static_arrays()

    wq = w_qkv[:, 0 * C:1 * C]
    wk = w_qkv[:, 1 * C:2 * C]
    wv = w_qkv[:, 2 * C:3 * C]

    in_maps = []
    for c in range(N_CORES):
        b = c // GROUPS
        hg = c % GROUPS
        hs = slice(hg * HPC * HD, (hg + 1) * HPC * HD)
        wq_c = wq[:, hs].reshape(C, HPC, HD)[:, :, perm].reshape(C, HPC * HD)
        wk_c = wk[:, hs].reshape(C, HPC, HD)[:, :, perm].reshape(C, HPC * HD)
        in_maps.append({
            "xT": np.ascontiguousarray(x[b].T).astype(np.float16),
            "wqk": np.concatenate([wq_c, wk_c], axis=1).astype(np.float16),
            "wv": np.ascontiguousarray(wv[:, hs]).astype(np.float16),
            "wp": np.ascontiguousarray(w_proj[hs, :]).astype(np.float16),
            "cosT": cosT,
            "sinT": sinT,
            "masks": masks,
        })
    return in_maps


class _PjrtRunner:
    """Caches the jitted shard_map callable so repeat kernel() calls skip
    retracing. Mirrors concourse.bass2jax.run_bass_via_pjrt."""

    def __init__(self, nc):
        import jax
        from jax.sharding import Mesh, PartitionSpec, NamedSharding
        from jax.experimental.shard_map import shard_map
        from concourse.bass2jax import (
            _bass_exec_p, install_neuronx_cc_hook, partition_id_tensor)

        install_neuronx_cc_hook()
        self.jax = jax
        partition_name = nc.partition_id_tensor.name if nc.partition_id_tensor else None
        in_names, out_names, out_avals = [], [], []
        for alloc in nc.m.functions[0].allocations:
            if not isinstance(alloc, mybir.MemoryLocationSet):
                continue
            name = alloc.memorylocations[0].name
            if alloc.kind == "ExternalInput":
                if name != partition_name:
                    in_names.append(name)
            elif alloc.kind == "ExternalOutput":
                out_names.append(name)
                out_avals.append(jax.core.ShapedArray(
                    tuple(alloc.tensor_shape), mybir.dt.np(alloc.dtype)))
        self.in_names, self.out_names, self.out_avals = in_names, out_names, out_avals
        n_params = len(in_names)
        n_outs = len(out_avals)
        bind_names = tuple(in_names + out_names +
                           ([partition_name] if partition_name else []))
        donate = tuple(range(n_params, n_params + n_outs))

        def _body(*args):
            operands = list(args)
            if partition_name:
                operands.append(partition_id_tensor())
            outs = _bass_exec_p.bind(
                *operands,
                out_avals=tuple(out_avals),
                in_names=bind_names,
                out_names=tuple(out_names),
                lowering_input_output_aliases=(),
                sim_require_finite=True,
                sim_require_nnan=True,
                nc=nc,
            )
            return tuple(outs)

        devices = jax.devices()[:N_CORES]
        self.mesh = Mesh(np.asarray(devices), ("core",))
        self.sharding = NamedSharding(self.mesh, PartitionSpec("core"))
        in_specs = (PartitionSpec("core"),) * (n_params + n_outs)
        out_specs = (PartitionSpec("core"),) * len(out_names)
        self.fn = jax.jit(
            shard_map(_body, mesh=self.mesh, in_specs=in_specs,
                      out_specs=out_specs, check_rep=False),
            donate_argnums=donate,
        )

    def run(self, in_maps):
        jax = self.jax
        concat = [
            np.concatenate([np.asarray(m[name]) for m in in_maps], axis=0)
            for name in self.in_names
        ]
        dev = [jax.device_put(c, self.sharding) for c in concat]
        zeros = [
            jax.device_put(
                np.zeros((N_CORES * a.shape[0], *a.shape[1:]), a.dtype),
                self.sharding)
            for a in self.out_avals
        ]
        outs = self.fn(*dev, *zeros)
        jax.block_until_ready(outs)
        res = []
        for c in range(N_CORES):
            d = {}
            for i, name in enumerate(self.out_names):
                a = np.asarray(outs[i])
                d[name] = a.reshape(N_CORES, *self.out_avals[i].shape)[c]
            res.append(d)
        return res


def _get_rt():
    if "rt" not in _CACHE:
        nc = _build_nc(T=T, C=C, HPC=HPC, n_cores=N_CORES, qn=QN, reps=1, an=AN)
        _CACHE["nc"] = nc
        _CACHE["rt"] = _PjrtRunner(nc) if axon_active() else None
    return _CACHE.get("nc"), _CACHE.get("rt")


def kernel(x, w_qkv, w_proj, n_head):
    assert int(n_head) == NH
    x = np.asarray(x, dtype=np.float32)
    w_qkv = np.asarray(w_qkv, dtype=np.float32)
    w_proj = np.asarray(w_proj, dtype=np.float32)
    assert x.shape == (B, T, C) and w_qkv.shape == (C, 3 * C) and w_proj.shape == (C, C)

    nc, rt = _get_rt()
    in_maps = _host_prep(x, w_qkv, w_proj)
    if rt is not None:
        results = rt.run(in_maps)
    else:
        results = run_bass_kernel_spmd(nc, in_maps,
                                       core_ids=list(range(N_CORES))).results

    out = np.zeros((B, T, C), dtype=np.float32)
    for c in range(N_CORES):
        b = c // GROUPS
        out[b] += results[c]["outT"].astype(np.float32).T
    return out



# revision 5
# speedup vs baseline: 1.3234x; 1.3234x over previous
"""Causal self-attention with RoPE (B=2, T=2048, C=2048, 16 heads) on 8 TRN2
NeuronCores.

Sharding: data-parallel over batch x tensor-parallel over heads.
Core c handles batch c//4 and heads 4*(c%4) .. 4*(c%4)+4. Each core computes
its heads' q/k/v projections, RoPE, causal attention, and a partial output
projection over its heads' channels; the host sums the 4 partial projections
per batch (the tensor-parallel reduce) and stacks the batches.

Per-core program (fp8 DoubleRow projections, fp16 attention, fp32 accum):
  A2: v[T, hd*4] = x.T @ wv as fp8e4m3 hi/lo 3-term (xh*wh + xh*wl + xl*wh)
      DoubleRow matmuls over 256-row contraction slabs; weights host-scaled
      x64, descale folded into the PSUM->SBUF copy (ACT, scale=1/64).
  A1: qT/kT[hd, T] = wqk.T @ x, same fp8 3-term DoubleRow; RoPE applied from
      the fp16 copy with fp16 tables (sign of sin folded into the table):
      dst = qraw*cos2 + rothalf(qraw)*sin2, all DVE.
  B:  per (q-tile, head): scoresT = k.T @ q in fp16 -> psS pairs; one exp per
      [128,1024] pair (ACT); diagonal masks post-exp (DVE); att accumulated
      into acc (DVE) for the denominator, finished by a single ones.T@acc
      matmul; y accumulates in PSUM; y/denom normalized and split into
      fp8 hi/lo (DVE) for the projection.
  C:  outT[C, T] partial = wp.T @ y8 as fp8 3-term DoubleRow, interleaved
      with the next q-tile's attention; psO evacuated by ACT/DVE copies
      (descale 1/64) and DMA'd out.
"""
import os
import numpy as np
from contextlib import ExitStack

os.environ.setdefault("JAX_COMPILATION_CACHE_DIR", "/tmp/jax_comp_cache")

import concourse.bass as bass
import concourse.tile as tile
from concourse import bacc, mybir
from concourse.bass_utils import run_bass_kernel_spmd
from concourse._compat import axon_active

FP16 = mybir.dt.float16
FP32 = mybir.dt.float32
FP8 = mybir.dt.float8e4

B, T, C, NH = 2, 2048, 2048, 16
HD = C // NH
N_CORES = 8
GROUPS = N_CORES // B
HPC = NH // GROUPS
QN = 512
AN = 1024
WS = 64.0                 # host weight scale (keeps fp8 out of subnormals)
SLAB = 256                # DoubleRow contraction slab
NSLAB = C // SLAB


def _build_nc(T=2048, C=2048, HPC=4, n_cores=8, qn=512, reps=1, an=AN):
    HD = 128
    QK_COLS = 2 * HPC * HD   # q+k columns per core (1024)
    V_COLS = HPC * HD        # v columns per core (512)
    NQT = T // qn            # q tiles in attention
    NKC = T // 128           # k chunks
    NSL = C // 256           # contraction slabs for DoubleRow
    scale = 1.0 / np.sqrt(np.float32(HD))
    DR = mybir.MatmulPerfMode.DoubleRow

    nc = bacc.Bacc("TRN2", target_bir_lowering=False, debug=False,
                   num_devices=n_cores)
    x8h_ap = nc.dram_tensor("x8h", (C, T), FP8, kind="ExternalInput").ap()
    x8l_ap = nc.dram_tensor("x8l", (C, T), FP8, kind="ExternalInput").ap()
    wqkh_ap = nc.dram_tensor("wqkh", (C, QK_COLS), FP8, kind="ExternalInput").ap()
    wqkl_ap = nc.dram_tensor("wqkl", (C, QK_COLS), FP8, kind="ExternalInput").ap()
    wvh_ap = nc.dram_tensor("wvh", (C, V_COLS), FP8, kind="ExternalInput").ap()
    wvl_ap = nc.dram_tensor("wvl", (C, V_COLS), FP8, kind="ExternalInput").ap()
    wph_ap = nc.dram_tensor("wph", (128, 4 * C), FP8, kind="ExternalInput").ap()
    wpl_ap = nc.dram_tensor("wpl", (128, 4 * C), FP8, kind="ExternalInput").ap()
    cos_ap = nc.dram_tensor("cos2", (128, T), FP16, kind="ExternalInput").ap()
    sin_ap = nc.dram_tensor("sin2", (128, T), FP16, kind="ExternalInput").ap()
    mask_ap = nc.dram_tensor("masks", (128, 4 * qn), FP16, kind="ExternalInput").ap()
    out_ap = nc.dram_tensor("outT", (C, T), FP16, kind="ExternalOutput").ap()

    x8h_r = x8h_ap.rearrange("(s p two) t -> p s two t", p=128, two=2)
    x8l_r = x8l_ap.rearrange("(s p two) t -> p s two t", p=128, two=2)
    wqkh_r = wqkh_ap.rearrange("(s p two) n -> p s two n", p=128, two=2)
    wqkl_r = wqkl_ap.rearrange("(s p two) n -> p s two n", p=128, two=2)
    wvh_r = wvh_ap.rearrange("(s p two) n -> p s two n", p=128, two=2)
    wvl_r = wvl_ap.rearrange("(s p two) n -> p s two n", p=128, two=2)

    with tile.TileContext(nc) as tc:
      for rep in range(reps):
        R = f"r{rep}_"
        with ExitStack() as top:
            xt_pool = top.enter_context(tc.tile_pool(name=R + "xt", bufs=1))
            qk_pool = top.enter_context(tc.tile_pool(name=R + "qk", bufs=1))
            v_pool = top.enter_context(tc.tile_pool(name=R + "v", bufs=1))
            tab_pool = top.enter_context(tc.tile_pool(name=R + "tab", bufs=1))

            xh = xt_pool.tile([128, NSL, 2, T], FP8)
            xl = xt_pool.tile([128, NSL, 2, T], FP8)
            qk_sb = qk_pool.tile([128, 2 * HPC, T], FP16)  # cols 0..3 q, 4..7 k
            v_sb = v_pool.tile([128, NKC, V_COLS], FP16)   # [t_lo, t_chunk, vcol]
            cos_t = tab_pool.tile([128, T], FP16)
            sin_t = tab_pool.tile([128, T], FP16)

            warm = tab_pool.tile([1, 1], FP32)
            nc.vector.memset(warm[:], 0.0)
            warm2 = tab_pool.tile([1, 1], FP32)
            nc.scalar.activation(warm2[:], warm[:],
                                 mybir.ActivationFunctionType.Exp)

            # ---- Phase A2 first: v projection (x DMAs pipeline with it) ----
            with ExitStack() as sV:
                wv_pool = sV.enter_context(tc.tile_pool(name=R + "wv", bufs=1))
                psV_pool = sV.enter_context(
                    tc.tile_pool(name=R + "psV", bufs=8, space="PSUM"))
                wvh = wv_pool.tile([128, NSL, 2, V_COLS], FP8)
                wvl = wv_pool.tile([128, NSL, 2, V_COLS], FP8)
                for s in range(NSL):
                    nc.sync.dma_start(xh[:, s, :, :], x8h_r[:, s, :, :])
                    nc.sync.dma_start(xl[:, s, :, :], x8l_r[:, s, :, :])
                    nc.sync.dma_start(wvh[:, s, :, :], wvh_r[:, s, :, :])
                    nc.sync.dma_start(wvl[:, s, :, :], wvl_r[:, s, :, :])
                for grp0 in range(0, NKC, 8):
                    psvs = []
                    for ts in range(grp0, grp0 + 8):
                        psv = psV_pool.tile([128, V_COLS], FP32, tag="psV",
                                            name=f"{R}psV_{ts}")
                        psvs.append(psv)
                    for s in range(NSL):
                        for gi, ts in enumerate(range(grp0, grp0 + 8)):
                            for term, (xs, ws) in enumerate(
                                    ((xh, wvh), (xh, wvl), (xl, wvh))):
                                nc.tensor.matmul(
                                    psvs[gi][:],
                                    xs[:, s, :, ts * 128:(ts + 1) * 128],
                                    ws[:, s, :, :],
                                    start=(s == 0 and term == 0),
                                    stop=(s == NSL - 1 and term == 2),
                                    perf_mode=DR)
                    for gi, ts in enumerate(range(grp0, grp0 + 8)):
                        nc.scalar.activation(
                            v_sb[:, ts, :], psvs[gi][:],
                            mybir.ActivationFunctionType.Copy,
                            scale=float(1.0 / WS))

            # ---- Phase A1: q/k projection + RoPE ----
            with ExitStack() as sA1:
                wqk_pool = sA1.enter_context(tc.tile_pool(name=R + "wqk", bufs=1))
                tmp_pool = sA1.enter_context(tc.tile_pool(name=R + "tmp", bufs=2))
                psA_pool = sA1.enter_context(
                    tc.tile_pool(name=R + "psA", bufs=8, space="PSUM"))
                wqh = wqk_pool.tile([128, NSL, 2, QK_COLS], FP8)
                wql = wqk_pool.tile([128, NSL, 2, QK_COLS], FP8)
                for s in range(NSL):
                    nc.sync.dma_start(wqh[:, s, :, :], wqkh_r[:, s, :, :])
                    nc.sync.dma_start(wql[:, s, :, :], wqkl_r[:, s, :, :])
                nc.sync.dma_start(cos_t[:], cos_ap[:])
                nc.sync.dma_start(sin_t[:], sin_ap[:])

                tiles = [(col, tq) for col in range(2 * HPC)
                         for tq in range(T // qn)]
                for g0 in range(0, len(tiles), 8):
                    grp = tiles[g0:g0 + 8]
                    pss = []
                    for (col, tq) in grp:
                        ps = psA_pool.tile([128, qn], FP32, tag="psA",
                                           name=f"{R}psA_{col}_{tq}")
                        pss.append(ps)
                    for s in range(NSL):
                        for gi, (col, tq) in enumerate(grp):
                            for term, (ws, xs) in enumerate(
                                    ((wqh, xh), (wqh, xl), (wql, xh))):
                                nc.tensor.matmul(
                                    pss[gi][:],
                                    ws[:, s, :, col * HD:(col + 1) * HD],
                                    xs[:, s, :, tq * qn:(tq + 1) * qn],
                                    start=(s == 0 and term == 0),
                                    stop=(s == NSL - 1 and term == 2),
                                    perf_mode=DR)
                    for gi, (col, tq) in enumerate(grp):
                        ts0, ts1 = tq * qn, (tq + 1) * qn
                        qraw = tmp_pool.tile([128, qn], FP16, tag="qraw",
                                             name=f"{R}qraw_{col}_{tq}")
                        nc.scalar.activation(
                            qraw[:], pss[gi][:],
                            mybir.ActivationFunctionType.Copy,
                            scale=float(1.0 / WS))
                        tcos = tmp_pool.tile([128, qn], FP16, tag="tcos",
                                             name=f"{R}tcos_{col}_{tq}")
                        tsin = tmp_pool.tile([128, qn], FP16, tag="tsin",
                                             name=f"{R}tsin_{col}_{tq}")
                        nc.vector.tensor_mul(tcos[:], qraw[:], cos_t[:, ts0:ts1])
                        nc.vector.tensor_mul(tsin[0:64, :], qraw[64:128, :],
                                             sin_t[64:128, ts0:ts1])
                        nc.vector.tensor_mul(tsin[64:128, :], qraw[0:64, :],
                                             sin_t[0:64, ts0:ts1])
                        nc.vector.tensor_add(qk_sb[:, col, ts0:ts1],
                                             tcos[:], tsin[:])

            # ---- Phase B + C: attention + projection, pipelined per q-tile --
            with ExitStack() as sB:
                msk_pool = sB.enter_context(tc.tile_pool(name=R + "msk", bufs=1))
                one_pool = sB.enter_context(tc.tile_pool(name=R + "one", bufs=1))
                att_pool = sB.enter_context(tc.tile_pool(name=R + "att", bufs=4))
                acc_pool = sB.enter_context(tc.tile_pool(name=R + "acc", bufs=2))
                rec_pool = sB.enter_context(tc.tile_pool(name=R + "rec", bufs=2))
                y8_pool = sB.enter_context(tc.tile_pool(name=R + "y8", bufs=1))
                wp_pool = sB.enter_context(tc.tile_pool(name=R + "wp", bufs=1))
                o_pool = sB.enter_context(tc.tile_pool(name=R + "o", bufs=4))
                psS_pool = sB.enter_context(
                    tc.tile_pool(name=R + "psS", bufs=2, space="PSUM"))
                psY_pool = sB.enter_context(
                    tc.tile_pool(name=R + "psY", bufs=2, space="PSUM"))
                psO_pool = sB.enter_context(
                    tc.tile_pool(name=R + "psO", bufs=2, space="PSUM"))

                msk = msk_pool.tile([128, 4, qn], FP16)
                nc.sync.dma_start(msk[:], mask_ap.rearrange("p (j n) -> p j n", n=qn))
                ones_t = one_pool.tile([128, 1], FP16)
                nc.vector.memset(ones_t[:], 1.0)
                wph = wp_pool.tile([128, 2, 2, C], FP8)
                wpl = wp_pool.tile([128, 2, 2, C], FP8)
                nc.sync.dma_start(wph[:], wph_ap.rearrange("p (s i c) -> p s i c", s=2, i=2))
                nc.sync.dma_start(wpl[:], wpl_ap.rearrange("p (s i c) -> p s i c", s=2, i=2))
                y8h = y8_pool.tile([128, HPC, T], FP8)
                y8l = y8_pool.tile([128, HPC, T], FP8)

                NCT = C // 128
                diag_per_qt = qn // 128

                def emit_scores(qt, h, j, ps):
                    """Two 128-k-chunk score matmuls into psS halves."""
                    for half in range(2):
                        kc = 2 * j + half
                        nc.tensor.matmul(
                            ps[:, half, :],
                            qk_sb[:, HPC + h, kc * 128:(kc + 1) * 128],
                            qk_sb[:, h, qt * qn:(qt + 1) * qn],
                            start=True, stop=True)

                def emit_unit_post(qt, h, j, np2, ps, att, acc, psY):
                    """exp + mask + denominator-accumulate + y matmuls."""
                    nc.scalar.activation(
                        att[:, :, :], ps[:, :, :],
                        mybir.ActivationFunctionType.Exp, scale=float(scale))
                    for half in range(2):
                        kc = 2 * j + half
                        jj = kc - diag_per_qt * qt
                        if jj >= 0:
                            nc.vector.tensor_mul(att[:, half, :], att[:, half, :],
                                                 msk[:, jj, :])
                    if j == 0:
                        nc.vector.tensor_copy(acc[:, :, :], att[:, :, :])
                    else:
                        nc.vector.tensor_add(acc[:, :, :], acc[:, :, :],
                                             att[:, :, :])
                    for half in range(2):
                        kc = 2 * j + half
                        nc.tensor.matmul(
                            psY[:], v_sb[:, kc, h * HD:(h + 1) * HD],
                            att[:, half, :],
                            start=(kc == 0), stop=(kc == 2 * np2 - 1))

                def emit_norm(qt, h, acc, psY):
                    """denominator finish + y normalize + fp8 hi/lo split."""
                    accf = acc_pool.tile([128, qn], FP16, tag="accf",
                                         name=f"{R}accf_{h}_{qt}")
                    nc.vector.tensor_add(accf[:], acc[:, 0, :], acc[:, 1, :])
                    psD = psO_pool.tile([128, qn], FP32, tag="psO",
                                        name=f"{R}psD_{h}_{qt}")
                    nc.tensor.matmul(psD[0:1, :], ones_t[:], accf[:],
                                     start=True, stop=True)
                    rec = rec_pool.tile([1, qn], FP32, tag="rec",
                                        name=f"{R}rec_{h}_{qt}")
                    nc.vector.reciprocal(rec[:], psD[0:1, :])
                    recb = rec_pool.tile([128, qn], FP32, tag="recb",
                                         name=f"{R}recb_{h}_{qt}")
                    nc.gpsimd.partition_broadcast(recb[:], rec[:])
                    yt = rec_pool.tile([128, qn], FP32, tag="yt",
                                       name=f"{R}yt_{h}_{qt}")
                    nc.vector.tensor_mul(yt[:], psY[:], recb[:])
                    ts0, ts1 = qt * qn, (qt + 1) * qn
                    nc.vector.tensor_copy(y8h[:, h, ts0:ts1], yt[:])
                    nc.vector.tensor_sub(y8l[:, h, ts0:ts1], yt[:],
                                         y8h[:, h, ts0:ts1])

                def emit_proj_ct(qt, ct, evac_dve):
                    pso = psO_pool.tile([128, 512], FP32, tag="psO",
                                        name=f"{R}psO_{ct}_{qt}")
                    n = 0
                    for s in range(2):
                        for (wp_, y8_) in ((wph, y8h), (wph, y8l), (wpl, y8h)):
                            nc.tensor.matmul(
                                pso[:], wp_[:, s, :, ct * 128:(ct + 1) * 128],
                                y8_[:, 2 * s:2 * s + 2, qt * qn:(qt + 1) * qn],
                                start=(n == 0), stop=(n == 5),
                                perf_mode=DR)
                            n += 1
                    o_t = o_pool.tile([128, 512], FP16, tag="o",
                                      name=f"{R}o_{ct}_{qt}")
                    if evac_dve:
                        nc.vector.tensor_scalar_mul(o_t[:], pso[:],
                                                    float(1.0 / WS))
                    else:
                        nc.scalar.activation(o_t[:], pso[:],
                                             mybir.ActivationFunctionType.Copy,
                                             scale=float(1.0 / WS))
                    nc.sync.dma_start(
                        out_ap[ct * 128:(ct + 1) * 128, qt * qn:(qt + 1) * qn],
                        o_t[:])

                # Global pipeline over (qt, h, j) units; proj of qt interleaved
                # into the start of qt+1's units.
                state = {}

                def make_unit(qt, h, j, np2):
                    ps = psS_pool.tile([128, 2, qn], FP32, tag="psS",
                                       name=f"{R}psS_{h}_{qt}_{j}")
                    emit_scores(qt, h, j, ps)
                    return ps

                all_units = []
                for qt in range(NQT):
                    np2 = diag_per_qt * (qt + 1) // 2
                    for h in range(HPC):
                        for j in range(np2):
                            all_units.append((qt, h, j, np2))

                # pending proj cts: emitted with a lag after their qt finishes
                pend_proj = []   # list of [due_idx, qt, ct, evac_dve]

                def queue_proj(qt, after_idx):
                    for ct in range(NCT):
                        evac_dve = (ct % 2 == 0)
                        # stagger: 4 units of lookahead before first ct
                        pend_proj.append([after_idx + 4 + ct // 4, qt, ct,
                                          evac_dve])

                prev_ps = None
                prev_meta = None
                for i, (qt, h, j, np2) in enumerate(all_units):
                    # emit any due proj work first (PE program order)
                    while pend_proj and pend_proj[0][0] <= i:
                        _, pqt, pct, pdve = pend_proj.pop(0)
                        emit_proj_ct(pqt, pct, pdve)
                    if j == 0:
                        state[(qt, h)] = {
                            "att_next": None,
                            "acc": acc_pool.tile([128, 2, qn], FP16, tag="acc",
                                                 name=f"{R}acc_{h}_{qt}"),
                            "psY": psY_pool.tile([128, qn], FP32, tag="psY",
                                                 name=f"{R}psY_{h}_{qt}"),
                        }
                    ps = make_unit(qt, h, j, np2)
                    if prev_ps is not None:
                        pqt, ph, pj, pnp2 = prev_meta
                        st = state[(pqt, ph)]
                        att = att_pool.tile([128, 2, qn], FP16, tag="att",
                                            name=f"{R}att_{ph}_{pqt}_{pj}")
                        emit_unit_post(pqt, ph, pj, pnp2, prev_ps, att,
                                       st["acc"], st["psY"])
                        if pj == pnp2 - 1:
                            emit_norm(pqt, ph, st["acc"], st["psY"])
                            if ph == HPC - 1:
                                queue_proj(pqt, i)
                    prev_ps = ps
                    prev_meta = (qt, h, j, np2)
                # drain last unit
                pqt, ph, pj, pnp2 = prev_meta
                st = state[(pqt, ph)]
                att = att_pool.tile([128, 2, qn], FP16, tag="att",
                                    name=f"{R}att_{ph}_{pqt}_{pj}")
                emit_unit_post(pqt, ph, pj, pnp2, prev_ps, att,
                               st["acc"], st["psY"])
                emit_norm(pqt, ph, st["acc"], st["psY"])
                queue_proj(pqt, len(all_units))
                while pend_proj:
                    _, pqt, pct, pdve = pend_proj.pop(0)
                    emit_proj_ct(pqt, pct, pdve)
    nc.compile()
    return nc


_CACHE = {}


def _rope_tables_np(t_len, hd):
    inv_freq = 1.0 / (10000.0 ** (np.arange(0, hd, 2, dtype=np.float32) / hd))
    t = np.arange(t_len, dtype=np.float32)
    freqs = np.outer(t, inv_freq)
    emb = np.concatenate([freqs, freqs], axis=-1)
    return np.cos(emb)[:, ::2].astype(np.float32), np.sin(emb)[:, ::2].astype(np.float32)


def _static_arrays():
    if "static" not in _CACHE:
        cos_, sin_ = _rope_tables_np(T, HD)
        cosT = np.ascontiguousarray(cos_.T)   # (64, T)
        sinT = np.ascontiguousarray(sin_.T)
        # sin table halves are indexed by the *source* partition of the
        # rotate-half read: rows 64..127 hold -sin (multiplies x2 into the
        # low half), rows 0..63 hold +sin (multiplies x1 into the high half).
        cos2 = np.concatenate([cosT, cosT], axis=0).astype(np.float16)
        sin2 = np.concatenate([sinT, -sinT], axis=0).astype(np.float16)
        perm = np.concatenate([np.arange(0, HD, 2), np.arange(1, HD, 2)])
        p = np.arange(128)[:, None]
        f = np.arange(QN)[None, :]
        masks = np.concatenate(
            [(p <= (f - 128 * j)).astype(np.float16) for j in range(QN // 128)],
            axis=1)
        _CACHE["static"] = (cos2, sin2, perm, masks)
    return _CACHE["static"]


def _q8(a, np8):
    hi = a.astype(np8)
    lo = (a - hi.astype(np.float32)).astype(np8)
    return hi, lo


def _host_prep(x, w_qkv, w_proj):
    cos2, sin2, perm, masks = _static_arrays()
    np8 = mybir.dt.np(FP8)

    wq = w_qkv[:, 0 * C:1 * C]
    wk = w_qkv[:, 1 * C:2 * C]
    wv = w_qkv[:, 2 * C:3 * C]

    in_maps = []
    xq = {}
    for b in range(B):
        xT = np.ascontiguousarray(x[b].T)
        xq[b] = _q8(xT, np8)
    for c in range(N_CORES):
        b = c // GROUPS
        hg = c % GROUPS
        hs = slice(hg * HPC * HD, (hg + 1) * HPC * HD)
        wq_c = wq[:, hs].reshape(C, HPC, HD)[:, :, perm].reshape(C, HPC * HD)
        wk_c = wk[:, hs].reshape(C, HPC, HD)[:, :, perm].reshape(C, HPC * HD)
        wqk_c = np.concatenate([wq_c, wk_c], axis=1) * WS
        wqkh, wqkl = _q8(wqk_c, np8)
        wvh, wvl = _q8(wv[:, hs] * WS, np8)
        # wp arranged [d, s, i, c]: row d holds heads (2s+i) of this group
        wp_c = (w_proj[hs, :] * WS).reshape(2, 2, 128, C).transpose(2, 0, 1, 3)
        wph, wpl = _q8(np.ascontiguousarray(wp_c.reshape(128, 4 * C)), np8)
        x8h, x8l = xq[b]
        in_maps.append({
            "x8h": x8h, "x8l": x8l,
            "wqkh": wqkh, "wqkl": wqkl,
            "wvh": wvh, "wvl": wvl,
            "wph": wph, "wpl": wpl,
            "cos2": cos2, "sin2": sin2,
            "masks": masks,
        })
    return in_maps


class _PjrtRunner:
    """Caches the jitted shard_map callable so repeat kernel() calls skip
    retracing. Mirrors concourse.bass2jax.run_bass_via_pjrt."""

    def __init__(self, nc):
        import jax
        from jax.sharding import Mesh, PartitionSpec, NamedSharding
        from jax.experimental.shard_map import shard_map
        from concourse.bass2jax import (
            _bass_exec_p, install_neuronx_cc_hook, partition_id_tensor)

        install_neuronx_cc_hook()
        self.jax = jax
        partition_name = nc.partition_id_tensor.name if nc.partition_id_tensor else None
        in_names, out_names, out_avals = [], [], []
        for alloc in nc.m.functions[0].allocations:
            if not isinstance(alloc, mybir.MemoryLocationSet):
                continue
            name = alloc.memorylocations[0].name
            if alloc.kind == "ExternalInput":
                if name != partition_name:
                    in_names.append(name)
            elif alloc.kind == "ExternalOutput":
                out_names.append(name)
                out_avals.append(jax.core.ShapedArray(
                    tuple(alloc.tensor_shape), mybir.dt.np(alloc.dtype)))
        self.in_names, self.out_names, self.out_avals = in_names, out_names, out_avals
        n_params = len(in_names)
        n_outs = len(out_avals)
        bind_names = tuple(in_names + out_names +
                           ([partition_name] if partition_name else []))
        donate = tuple(range(n_params, n_params + n_outs))

        def _body(*args):
            operands = list(args)
            if partition_name:
                operands.append(partition_id_tensor())
            outs = _bass_exec_p.bind(
                *operands,
                out_avals=tuple(out_avals),
                in_names=bind_names,
                out_names=tuple(out_names),
                lowering_input_output_aliases=(),
                sim_require_finite=True,
                sim_require_nnan=True,
                nc=nc,
            )
            return tuple(outs)

        devices = jax.devices()[:N_CORES]
        self.mesh = Mesh(np.asarray(devices), ("core",))
        self.sharding = NamedSharding(self.mesh, PartitionSpec("core"))
        in_specs = (PartitionSpec("core"),) * (n_params + n_outs)
        out_specs = (PartitionSpec("core"),) * len(out_names)
        self.fn = jax.jit(
            shard_map(_body, mesh=self.mesh, in_specs=in_specs,
                      out_specs=out_specs, check_rep=False),
            donate_argnums=donate,
        )

    def run(self, in_maps):
        jax = self.jax
        concat = [
            np.concatenate([np.asarray(m[name]) for m in in_maps], axis=0)
            for name in self.in_names
        ]
        dev = [jax.device_put(c, self.sharding) for c in concat]
        zeros = [
            jax.device_put(
                np.zeros((N_CORES * a.shape[0], *a.shape[1:]), a.dtype),
                self.sharding)
            for a in self.out_avals
        ]
        outs = self.fn(*dev, *zeros)
        jax.block_until_ready(outs)
        res = []
        for c in range(N_CORES):
            d = {}
            for i, name in enumerate(self.out_names):
                a = np.asarray(outs[i])
                d[name] = a.reshape(N_CORES, *self.out_avals[i].shape)[c]
            res.append(d)
        return res


def _get_rt():
    if "rt" not in _CACHE:
        nc = _build_nc(T=T, C=C, HPC=HPC, n_cores=N_CORES, qn=QN, reps=1, an=AN)
        _CACHE["nc"] = nc
        _CACHE["rt"] = _PjrtRunner(nc) if axon_active() else None
    return _CACHE.get("nc"), _CACHE.get("rt")


def kernel(x, w_qkv, w_proj, n_head):
    assert int(n_head) == NH
    x = np.asarray(x, dtype=np.float32)
    w_qkv = np.asarray(w_qkv, dtype=np.float32)
    w_proj = np.asarray(w_proj, dtype=np.float32)
    assert x.shape == (B, T, C) and w_qkv.shape == (C, 3 * C) and w_proj.shape == (C, C)

    nc, rt = _get_rt()
    in_maps = _host_prep(x, w_qkv, w_proj)
    if rt is not None:
        results = rt.run(in_maps)
    else:
        results = run_bass_kernel_spmd(nc, in_maps,
                                       core_ids=list(range(N_CORES))).results

    out = np.zeros((B, T, C), dtype=np.float32)
    for c in range(N_CORES):
        b = c // GROUPS
        out[b] += results[c]["outT"].astype(np.float32).T
    return out


# revision 19
# speedup vs baseline: 1.3995x; 1.0575x over previous
"""Causal self-attention with RoPE (B=2, T=2048, C=2048, 16 heads) on 8 TRN2
NeuronCores.

Sharding: data-parallel over batch x tensor-parallel over heads.
Core c handles batch c//4 and heads 4*(c%4) .. 4*(c%4)+4. Each core computes
its heads' q/k/v projections, RoPE, causal attention, and a partial output
projection over its heads' channels; the host sums the 4 partial projections
per batch (the tensor-parallel reduce) and stacks the batches.

Per-core program (fp8 DoubleRow projections, fp16 attention, fp32 accum):
  A2: v[T, hd*4] = x.T @ wv as fp8e4m3 hi/lo 3-term (xh*wh + xh*wl + xl*wh)
      DoubleRow matmuls over 256-row contraction slabs; weights host-scaled
      x64, descale folded into the PSUM->SBUF copy (ACT, scale=1/64).
  A1: qT/kT[hd, T] = wqk.T @ x, same fp8 3-term DoubleRow; RoPE applied from
      the fp16 copy with fp16 tables (sign of sin folded into the table):
      dst = qraw*cos2 + rothalf(qraw)*sin2, all DVE.
  B:  per (q-tile, head): scoresT = k.T @ q in fp16 -> psS pairs; one exp per
      [128,1024] pair (ACT); diagonal masks post-exp (DVE); att accumulated
      into acc (DVE) for the denominator, finished by a single ones.T@acc
      matmul; y accumulates in PSUM; y/denom normalized and split into
      fp8 hi/lo (DVE) for the projection.
  C:  outT[C, T] partial = wp.T @ y8 as fp8 3-term DoubleRow, interleaved
      with the next q-tile's attention; psO evacuated by ACT/DVE copies
      (descale 1/64) and DMA'd out.
"""
import os
import numpy as np
from contextlib import ExitStack

os.environ.setdefault("JAX_COMPILATION_CACHE_DIR", "/tmp/jax_comp_cache")

import concourse.bass as bass
import concourse.tile as tile
from concourse import bacc, mybir
from concourse.bass_utils import run_bass_kernel_spmd
from concourse._compat import axon_active

FP16 = mybir.dt.float16
FP32 = mybir.dt.float32
FP8 = mybir.dt.float8e4

B, T, C, NH = 2, 2048, 2048, 16
HD = C // NH
N_CORES = 8
GROUPS = N_CORES // B
HPC = NH // GROUPS
QN = 512
AN = 1024
WS = 64.0                 # host weight scale (keeps fp8 out of subnormals)
SLAB = 256                # DoubleRow contraction slab
NSLAB = C // SLAB


def _build_nc(T=2048, C=2048, HPC=4, n_cores=8, qn=512, reps=1, an=AN):
    HD = 128
    QK_COLS = 2 * HPC * HD   # q+k columns per core (1024)
    V_COLS = HPC * HD        # v columns per core (512)
    NQT = T // qn            # q tiles in attention
    NKC = T // 128           # k chunks
    NSL = C // 256           # contraction slabs for DoubleRow
    scale = 1.0 / np.sqrt(np.float32(HD))
    DR = mybir.MatmulPerfMode.DoubleRow

    nc = bacc.Bacc("TRN2", target_bir_lowering=False, debug=False,
                   num_devices=n_cores)
    x8h_ap = nc.dram_tensor("x8h", (C, T), FP8, kind="ExternalInput").ap()
    x8l_ap = nc.dram_tensor("x8l", (C, T), FP8, kind="ExternalInput").ap()
    wqkh_ap = nc.dram_tensor("wqkh", (C, QK_COLS), FP8, kind="ExternalInput").ap()
    wqkl_ap = nc.dram_tensor("wqkl", (C, QK_COLS), FP8, kind="ExternalInput").ap()
    wvh_ap = nc.dram_tensor("wvh", (C, V_COLS), FP8, kind="ExternalInput").ap()
    wvl_ap = nc.dram_tensor("wvl", (C, V_COLS), FP8, kind="ExternalInput").ap()
    wph_ap = nc.dram_tensor("wph", (128, 4 * C), FP8, kind="ExternalInput").ap()
    wpl_ap = nc.dram_tensor("wpl", (128, 4 * C), FP8, kind="ExternalInput").ap()
    cos_ap = nc.dram_tensor("cos2", (128, T), FP16, kind="ExternalInput").ap()
    sin_ap = nc.dram_tensor("sin2", (128, T), FP16, kind="ExternalInput").ap()
    mask_ap = nc.dram_tensor("masks", (128, 4 * qn), FP16, kind="ExternalInput").ap()
    out_ap = nc.dram_tensor("outT", (C, T), FP16, kind="ExternalOutput").ap()

    x8h_r = x8h_ap.rearrange("(s p two) t -> p s two t", p=128, two=2)
    x8l_r = x8l_ap.rearrange("(s p two) t -> p s two t", p=128, two=2)
    wqkh_r = wqkh_ap.rearrange("(s p two) n -> p s two n", p=128, two=2)
    wqkl_r = wqkl_ap.rearrange("(s p two) n -> p s two n", p=128, two=2)
    wvh_r = wvh_ap.rearrange("(s p two) n -> p s two n", p=128, two=2)
    wvl_r = wvl_ap.rearrange("(s p two) n -> p s two n", p=128, two=2)

    with tile.TileContext(nc) as tc:
      for rep in range(reps):
        R = f"r{rep}_"
        with ExitStack() as top:
            xt_pool = top.enter_context(tc.tile_pool(name=R + "xt", bufs=1))
            qk_pool = top.enter_context(tc.tile_pool(name=R + "qk", bufs=1))
            v_pool = top.enter_context(tc.tile_pool(name=R + "v", bufs=1))
            tab_pool = top.enter_context(tc.tile_pool(name=R + "tab", bufs=1))

            xh = xt_pool.tile([128, NSL, 2, T], FP8)
            xl = xt_pool.tile([128, NSL, 2, T], FP8)
            qk_sb = qk_pool.tile([128, 2 * HPC, T], FP16)  # cols 0..3 q, 4..7 k
            v_sb = v_pool.tile([128, NKC, V_COLS], FP16)   # [t_lo, t_chunk, vcol]
            cos_t = tab_pool.tile([128, T], FP16)
            sin_t = tab_pool.tile([128, T], FP16)
            msk = tab_pool.tile([128, 4, qn], FP16)
            ones_t = tab_pool.tile([128, 1], FP16)
            wph = tab_pool.tile([128, 2, 2, C], FP8)
            wpl = tab_pool.tile([128, 2, 2, C], FP8)

            warm = tab_pool.tile([1, 1], FP32)
            nc.vector.memset(warm[:], 0.0)
            nc.vector.memset(ones_t[:], 1.0)
            warm2 = tab_pool.tile([1, 1], FP32)
            nc.scalar.activation(warm2[:], warm[:],
                                 mybir.ActivationFunctionType.Exp)

            # ---- Phase A2: v projection (x DMAs pipeline into group 0) ----
            TH = T // 2
            with ExitStack() as sV:
                wv_pool = sV.enter_context(tc.tile_pool(name=R + "wv", bufs=1))
                psV_pool = sV.enter_context(
                    tc.tile_pool(name=R + "psV", bufs=8, space="PSUM"))
                wvh = wv_pool.tile([128, NSL, 2, V_COLS], FP8)
                wvl = wv_pool.tile([128, NSL, 2, V_COLS], FP8)
                # DMA order = consumption order: group 0 needs x T0 + wv
                # slab-by-slab; then group 1's x T1 interleaved with wqk
                # (A1 g0 starts right after A2); tables/masks next (RoPE and
                # the first attention units); wp last (first proj ~30us in).
                for s in range(NSL):
                    nc.sync.dma_start(xh[:, s, :, 0:TH], x8h_r[:, s, :, 0:TH])
                    nc.sync.dma_start(wvh[:, s, :, :], wvh_r[:, s, :, :])
                    nc.sync.dma_start(wvl[:, s, :, :], wvl_r[:, s, :, :])
                for s in range(NSL):
                    nc.sync.dma_start(xh[:, s, :, TH:T], x8h_r[:, s, :, TH:T])
                for s in range(NSL):
                    nc.sync.dma_start(xl[:, s, :, 0:TH], x8l_r[:, s, :, 0:TH])
                for s in range(NSL):
                    nc.sync.dma_start(xl[:, s, :, TH:T], x8l_r[:, s, :, TH:T])
                for grp0 in range(0, NKC, 8):
                    psvs = []
                    for ts in range(grp0, grp0 + 8):
                        psv = psV_pool.tile([128, V_COLS], FP32, tag="psV",
                                            name=f"{R}psV_{ts}")
                        psvs.append(psv)
                    for phase, terms in enumerate((((xh, wvh), (xh, wvl)),
                                                   ((xl, wvh),))):
                      for s in range(NSL):
                        for gi, ts in enumerate(range(grp0, grp0 + 8)):
                            for (xs, ws) in terms:
                                nc.tensor.matmul(
                                    psvs[gi][:],
                                    xs[:, s, :, ts * 128:(ts + 1) * 128],
                                    ws[:, s, :, :],
                                    start=(phase == 0 and s == 0
                                           and ws is wvh),
                                    stop=(phase == 1 and s == NSL - 1),
                                    perf_mode=DR)
                            if phase == 1 and s == NSL - 1:
                                # alternate engines so the trailing copies
                                # drain in parallel (PSUM handover gate)
                                if ts % 2 == 0:
                                    nc.scalar.activation(
                                        v_sb[:, ts, :], psvs[gi][:],
                                        mybir.ActivationFunctionType.Copy,
                                        scale=float(1.0 / WS))
                                else:
                                    nc.vector.tensor_scalar_mul(
                                        v_sb[:, ts, :], psvs[gi][:],
                                        float(1.0 / WS))

            # ---- Merged A1 (q/k proj + RoPE) and B/C (attention + proj) ----
            with ExitStack() as sM:
                wqk_pool = sM.enter_context(tc.tile_pool(name=R + "wqk", bufs=1))
                tmp_pool = sM.enter_context(tc.tile_pool(name=R + "tmp", bufs=2))
                att_pool = sM.enter_context(tc.tile_pool(name=R + "att", bufs=3))
                acc_pool = sM.enter_context(tc.tile_pool(name=R + "acc", bufs=2))
                rec_pool = sM.enter_context(tc.tile_pool(name=R + "rec", bufs=2))
                y8_pool = sM.enter_context(tc.tile_pool(name=R + "y8", bufs=2))
                o_pool = sM.enter_context(tc.tile_pool(name=R + "o", bufs=3))
                psS_pool = sM.enter_context(
                    tc.tile_pool(name=R + "psS", bufs=2, space="PSUM"))
                psY_pool = sM.enter_context(
                    tc.tile_pool(name=R + "psY", bufs=2, space="PSUM"))
                psO_pool = sM.enter_context(
                    tc.tile_pool(name=R + "psO", bufs=2, space="PSUM"))

                wqh = wqk_pool.tile([128, NSL, 2, QK_COLS], FP8)
                wql = wqk_pool.tile([128, NSL, 2, QK_COLS], FP8)
                for s in range(NSL):
                    nc.sync.dma_start(wqh[:, s, :, :], wqkh_r[:, s, :, :])
                    nc.sync.dma_start(wql[:, s, :, :], wqkl_r[:, s, :, :])
                nc.sync.dma_start(cos_t[:], cos_ap[:])
                nc.sync.dma_start(sin_t[:], sin_ap[:])
                nc.sync.dma_start(msk[:], mask_ap.rearrange("p (j n) -> p j n", n=qn))
                nc.sync.dma_start(wph[:], wph_ap.rearrange("p (s i c) -> p s i c", s=2, i=2))
                nc.sync.dma_start(wpl[:], wpl_ap.rearrange("p (s i c) -> p s i c", s=2, i=2))


                NCT = C // 128
                diag_per_qt = qn // 128

                def emit_a1_tile(col, tq, ps_half):
                    """24 DoubleRow matmuls into one psS half, then RoPE."""
                    for s in range(NSL):
                        for term, (ws, xs) in enumerate(
                                ((wqh, xh), (wqh, xl), (wql, xh))):
                            nc.tensor.matmul(
                                ps_half,
                                ws[:, s, :, col * HD:(col + 1) * HD],
                                xs[:, s, :, tq * qn:(tq + 1) * qn],
                                start=(s == 0 and term == 0),
                                stop=(s == NSL - 1 and term == 2),
                                perf_mode=DR)

                def emit_rope(col, tq, qraw):
                    ts0, ts1 = tq * qn, (tq + 1) * qn
                    tcos = tmp_pool.tile([128, qn], FP16, tag="tcos", bufs=1,
                                         name=f"{R}tcos_{col}_{tq}")
                    tsin = tmp_pool.tile([128, qn], FP16, tag="tsin", bufs=1,
                                         name=f"{R}tsin_{col}_{tq}")
                    nc.vector.tensor_mul(tcos[:], qraw[:], cos_t[:, ts0:ts1])
                    nc.vector.tensor_mul(tsin[0:64, :], qraw[64:128, :],
                                         sin_t[64:128, ts0:ts1])
                    nc.vector.tensor_mul(tsin[64:128, :], qraw[0:64, :],
                                         sin_t[0:64, ts0:ts1])
                    nc.vector.tensor_add(qk_sb[:, col, ts0:ts1],
                                         tcos[:], tsin[:])

                def pair_qoff(qt, j):
                    # diag pair whose two chunks mask off q < 256 entirely
                    return 256 if (2 * j - diag_per_qt * qt) >= 2 else 0

                def emit_scores(qt, h, j, ps):
                    qoff = pair_qoff(qt, j)
                    for half in range(2):
                        kc = 2 * j + half
                        nc.tensor.matmul(
                            ps[:, half, qoff:],
                            qk_sb[:, HPC + h, kc * 128:(kc + 1) * 128],
                            qk_sb[:, h, qt * qn + qoff:(qt + 1) * qn],
                            start=True, stop=True)

                def emit_unit_post(qt, h, j, np2, ps, att, acc, psY):
                    qoff = pair_qoff(qt, j)
                    nc.scalar.activation(
                        att[:, :, qoff:], ps[:, :, qoff:],
                        mybir.ActivationFunctionType.Exp, scale=float(scale))
                    for half in range(2):
                        kc = 2 * j + half
                        jj = kc - diag_per_qt * qt
                        if jj >= 0:
                            nc.vector.tensor_mul(att[:, half, qoff:],
                                                 att[:, half, qoff:],
                                                 msk[:, jj, qoff:])
                    if j == 0:
                        nc.vector.tensor_copy(acc[:, :, :], att[:, :, :])
                    else:
                        nc.vector.tensor_add(acc[:, :, qoff:], acc[:, :, qoff:],
                                             att[:, :, qoff:])
                    for half in range(2):
                        kc = 2 * j + half
                        nc.tensor.matmul(
                            psY[:, qoff:], v_sb[:, kc, h * HD:(h + 1) * HD],
                            att[:, half, qoff:],
                            start=(kc == 0), stop=(kc == 2 * np2 - 1))

                def emit_norm(qt, h, acc, psY):
                    accf = acc_pool.tile([128, qn], FP16, tag="accf", bufs=1,
                                         name=f"{R}accf_{h}_{qt}")
                    nc.vector.tensor_add(accf[:], acc[:, 0, :], acc[:, 1, :])
                    psD = psO_pool.tile([128, qn], FP32, tag="psO",
                                        name=f"{R}psD_{h}_{qt}")
                    nc.tensor.matmul(psD[0:1, :], ones_t[:], accf[:],
                                     start=True, stop=True)
                    rec = rec_pool.tile([1, qn], FP32, tag="rec", bufs=1,
                                        name=f"{R}rec_{h}_{qt}")
                    nc.vector.reciprocal(rec[:], psD[0:1, :])
                    recb = rec_pool.tile([128, qn], FP32, tag="recb",
                                         name=f"{R}recb_{h}_{qt}")
                    nc.gpsimd.partition_broadcast(recb[:], rec[:])
                    yt = rec_pool.tile([128, qn], FP32, tag="yt", bufs=1,
                                       name=f"{R}yt_{h}_{qt}")
                    nc.vector.tensor_mul(yt[:], psY[:], recb[:])
                    y8h, y8l = y8_of_qt[qt]
                    nc.vector.tensor_copy(y8h[:, h, :], yt[:])
                    nc.vector.tensor_sub(y8l[:, h, :], yt[:], y8h[:, h, :])

                def emit_proj_ct(qt, ct, evac_dve, use_psS=False):
                    y8h, y8l = y8_of_qt[qt]
                    if use_psS:
                        psw = psS_pool.tile([128, 2, qn], FP32, tag="psS",
                                            name=f"{R}psOS_{ct}_{qt}")
                        pso = psw[:, 0, :]
                    else:
                        pso = psO_pool.tile([128, 512], FP32, tag="psO",
                                            name=f"{R}psO_{ct}_{qt}")
                    n = 0
                    for s in range(2):
                        for (wp_, y8_) in ((wph, y8h), (wph, y8l), (wpl, y8h)):
                            nc.tensor.matmul(
                                pso[:], wp_[:, s, :, ct * 128:(ct + 1) * 128],
                                y8_[:, 2 * s:2 * s + 2, :],
                                start=(n == 0), stop=(n == 5),
                                perf_mode=DR)
                            n += 1
                    o_t = o_pool.tile([128, 512], FP16, tag="o",
                                      name=f"{R}o_{ct}_{qt}")
                    if evac_dve:
                        nc.vector.tensor_scalar_mul(o_t[:], pso[:],
                                                    float(1.0 / WS))
                    else:
                        nc.scalar.activation(o_t[:], pso[:],
                                             mybir.ActivationFunctionType.Copy,
                                             scale=float(1.0 / WS))
                    nc.sync.dma_start(
                        out_ap[ct * 128:(ct + 1) * 128, qt * qn:(qt + 1) * qn],
                        o_t[:])

                # global pipeline: per tq group, A1 pair-tiles and proj of
                # qt-1 interleaved into qt's units via a side-work queue.
                prev = [None, None]   # ps, meta

                def flush_prev(i_slot):
                    if prev[0] is None:
                        return
                    pqt, ph, pj, pnp2 = prev[1]
                    st = state[(pqt, ph)]
                    att = att_pool.tile([128, 2, qn], FP16, tag="att",
                                        name=f"{R}att_{ph}_{pqt}_{pj}")
                    emit_unit_post(pqt, ph, pj, pnp2, prev[0], att,
                                   st["acc"], st["psY"])
                    if pj == pnp2 - 1:
                        emit_norm(pqt, ph, st["acc"], st["psY"])
                    prev[0] = None

                state = {}
                y8_of_qt = {}
                side_q = []   # [due_slot, fn]
                slot = [0]

                def emit_a1_pair(g, h):
                    ps = psS_pool.tile([128, 2, qn], FP32, tag="psS",
                                       name=f"{R}psA_{g}_{h}")
                    emit_a1_tile(HPC + h, g, ps[:, 0, :])   # k col
                    emit_a1_tile(h, g, ps[:, 1, :])         # q col
                    qraw = tmp_pool.tile([128, 2, qn], FP16, tag="qraw",
                                         bufs=1, name=f"{R}qraw_{g}_{h}")
                    nc.scalar.activation(
                        qraw[:, :, :], ps[:, :, :],
                        mybir.ActivationFunctionType.Copy,
                        scale=float(1.0 / WS))
                    emit_rope(HPC + h, g, qraw[:, 0, :])
                    emit_rope(h, g, qraw[:, 1, :])

                # A1 group 0 block
                for h in range(HPC):
                    emit_a1_pair(0, h)

                for g in range(NQT):
                    np2 = diag_per_qt * (g + 1) // 2
                    U = HPC * np2
                    s0 = slot[0]
                    # pre-queue side work for this stretch of units
                    if g >= 1:
                        for ct in range(NCT):
                            side_q.append(
                                [s0 + 2 + ct // 4,
                                 (lambda pqt, pct, pdve:
                                  lambda: emit_proj_ct(pqt, pct, pdve))(
                                     g - 1, ct, ct % 2 == 0)])
                    side_q.sort(key=lambda it: it[0])
                    if g >= 1:
                        for h in range(HPC):
                            emit_a1_pair(g, h)
                    for h in range(HPC):
                        for j in range(np2):
                            if j == 0 and h == 0:
                                y8_of_qt[g] = (
                                    y8_pool.tile([128, HPC, qn], FP8,
                                                 tag="y8h", name=f"{R}y8h_{g}"),
                                    y8_pool.tile([128, HPC, qn], FP8,
                                                 tag="y8l", name=f"{R}y8l_{g}"),
                                )
                            if j == 0:
                                state[(g, h)] = {
                                    "acc": acc_pool.tile(
                                        [128, 2, qn], FP16, tag="acc",
                                        name=f"{R}acc_{h}_{g}"),
                                    "psY": psY_pool.tile(
                                        [128, qn], FP32, tag="psY",
                                        name=f"{R}psY_{h}_{g}"),
                                }
                            ps = psS_pool.tile([128, 2, qn], FP32, tag="psS",
                                               name=f"{R}psS_{h}_{g}_{j}")
                            emit_scores(g, h, j, ps)
                            flush_prev(slot[0])
                            prev[0] = ps
                            prev[1] = (g, h, j, np2)
                            while side_q and side_q[0][0] <= slot[0]:
                                side_q.pop(0)[1]()
                            slot[0] += 1
                    # group end: ensure last unit's consumers are emitted
                    # before the next allocations recycle its psS slot
                    flush_prev(slot[0])
                    while side_q:
                        side_q.pop(0)[1]()
                # final projection for the last q-tile; alternate PSUM tags
                # (psS slots are idle by now) for a deeper evac pipeline
                for ct in range(NCT):
                    emit_proj_ct(NQT - 1, ct, ct % 2 == 0, use_psS=(ct % 2 == 1))
    nc.compile()
    return nc


_CACHE = {}


def _rope_tables_np(t_len, hd):
    inv_freq = 1.0 / (10000.0 ** (np.arange(0, hd, 2, dtype=np.float32) / hd))
    t = np.arange(t_len, dtype=np.float32)
    freqs = np.outer(t, inv_freq)
    emb = np.concatenate([freqs, freqs], axis=-1)
    return np.cos(emb)[:, ::2].astype(np.float32), np.sin(emb)[:, ::2].astype(np.float32)


def _static_arrays():
    if "static" not in _CACHE:
        cos_, sin_ = _rope_tables_np(T, HD)
        cosT = np.ascontiguousarray(cos_.T)   # (64, T)
        sinT = np.ascontiguousarray(sin_.T)
        # sin table halves are indexed by the *source* partition of the
        # rotate-half read: rows 64..127 hold -sin (multiplies x2 into the
        # low half), rows 0..63 hold +sin (multiplies x1 into the high half).
        cos2 = np.concatenate([cosT, cosT], axis=0).astype(np.float16)
        sin2 = np.concatenate([sinT, -sinT], axis=0).astype(np.float16)
        perm = np.concatenate([np.arange(0, HD, 2), np.arange(1, HD, 2)])
        p = np.arange(128)[:, None]
        f = np.arange(QN)[None, :]
        masks = np.concatenate(
            [(p <= (f - 128 * j)).astype(np.float16) for j in range(QN // 128)],
            axis=1)
        _CACHE["static"] = (cos2, sin2, perm, masks)
    return _CACHE["static"]


def _q8(a, np8):
    hi = a.astype(np8)
    lo = (a - hi.astype(np.float32)).astype(np8)
    return hi, lo


def _host_prep(x, w_qkv, w_proj):
    cos2, sin2, perm, masks = _static_arrays()
    np8 = mybir.dt.np(FP8)

    wq = w_qkv[:, 0 * C:1 * C]
    wk = w_qkv[:, 1 * C:2 * C]
    wv = w_qkv[:, 2 * C:3 * C]

    in_maps = []
    xq = {}
    for b in range(B):
        xT = np.ascontiguousarray(x[b].T)
        xq[b] = _q8(xT, np8)
    for c in range(N_CORES):
        b = c // GROUPS
        hg = c % GROUPS
        hs = slice(hg * HPC * HD, (hg + 1) * HPC * HD)
        wq_c = wq[:, hs].reshape(C, HPC, HD)[:, :, perm].reshape(C, HPC * HD)
        wk_c = wk[:, hs].reshape(C, HPC, HD)[:, :, perm].reshape(C, HPC * HD)
        wqk_c = np.concatenate([wq_c, wk_c], axis=1) * WS
        wqkh, wqkl = _q8(wqk_c, np8)
        wvh, wvl = _q8(wv[:, hs] * WS, np8)
        # wp arranged [d, s, i, c]: row d holds heads (2s+i) of this group
        wp_c = (w_proj[hs, :] * WS).reshape(2, 2, 128, C).transpose(2, 0, 1, 3)
        wph, wpl = _q8(np.ascontiguousarray(wp_c.reshape(128, 4 * C)), np8)
        x8h, x8l = xq[b]
        in_maps.append({
            "x8h": x8h, "x8l": x8l,
            "wqkh": wqkh, "wqkl": wqkl,
            "wvh": wvh, "wvl": wvl,
            "wph": wph, "wpl": wpl,
            "cos2": cos2, "sin2": sin2,
            "masks": masks,
        })
    return in_maps


class _PjrtRunner:
    """Caches the jitted shard_map callable so repeat kernel() calls skip
    retracing. Mirrors concourse.bass2jax.run_bass_via_pjrt."""

    def __init__(self, nc):
        import jax
        from jax.sharding import Mesh, PartitionSpec, NamedSharding
        from jax.experimental.shard_map import shard_map
        from concourse.bass2jax import (
            _bass_exec_p, install_neuronx_cc_hook, partition_id_tensor)

        install_neuronx_cc_hook()
        self.jax = jax
        partition_name = nc.partition_id_tensor.name if nc.partition_id_tensor else None
        in_names, out_names, out_avals = [], [], []
        for alloc in nc.m.functions[0].allocations:
            if not isinstance(alloc, mybir.MemoryLocationSet):
                continue
            name = alloc.memorylocations[0].name
            if alloc.kind == "ExternalInput":
                if name != partition_name:
                    in_names.append(name)
            elif alloc.kind == "ExternalOutput":
                out_names.append(name)
                out_avals.append(jax.core.ShapedArray(
                    tuple(alloc.tensor_shape), mybir.dt.np(alloc.dtype)))
        self.in_names, self.out_names, self.out_avals = in_names, out_names, out_avals
        n_params = len(in_names)
        n_outs = len(out_avals)
        bind_names = tuple(in_names + out_names +
                           ([partition_name] if partition_name else []))
        donate = tuple(range(n_params, n_params + n_outs))

        def _body(*args):
            operands = list(args)
            if partition_name:
                operands.append(partition_id_tensor())
            outs = _bass_exec_p.bind(
                *operands,
                out_avals=tuple(out_avals),
                in_names=bind_names,
                out_names=tuple(out_names),
                lowering_input_output_aliases=(),
                sim_require_finite=True,
                sim_require_nnan=True,
                nc=nc,
            )
            return tuple(outs)

        devices = jax.devices()[:N_CORES]
        self.mesh = Mesh(np.asarray(devices), ("core",))
        self.sharding = NamedSharding(self.mesh, PartitionSpec("core"))
        in_specs = (PartitionSpec("core"),) * (n_params + n_outs)
        out_specs = (PartitionSpec("core"),) * len(out_names)
        self.fn = jax.jit(
            shard_map(_body, mesh=self.mesh, in_specs=in_specs,
                      out_specs=out_specs, check_rep=False),
            donate_argnums=donate,
        )

    def run(self, in_maps):
        jax = self.jax
        concat = [
            np.concatenate([np.asarray(m[name]) for m in in_maps], axis=0)
            for name in self.in_names
        ]
        dev = [jax.device_put(c, self.sharding) for c in concat]
        zeros = [
            jax.device_put(
                np.zeros((N_CORES * a.shape[0], *a.shape[1:]), a.dtype),
                self.sharding)
            for a in self.out_avals
        ]
        outs = self.fn(*dev, *zeros)
        jax.block_until_ready(outs)
        res = []
        for c in range(N_CORES):
            d = {}
            for i, name in enumerate(self.out_names):
                a = np.asarray(outs[i])
                d[name] = a.reshape(N_CORES, *self.out_avals[i].shape)[c]
            res.append(d)
        return res


def _get_rt():
    if "rt" not in _CACHE:
        nc = _build_nc(T=T, C=C, HPC=HPC, n_cores=N_CORES, qn=QN, reps=1, an=AN)
        _CACHE["nc"] = nc
        _CACHE["rt"] = _PjrtRunner(nc) if axon_active() else None
    return _CACHE.get("nc"), _CACHE.get("rt")


def kernel(x, w_qkv, w_proj, n_head):
    assert int(n_head) == NH
    x = np.asarray(x, dtype=np.float32)
    w_qkv = np.asarray(w_qkv, dtype=np.float32)
    w_proj = np.asarray(w_proj, dtype=np.float32)
    assert x.shape == (B, T, C) and w_qkv.shape == (C, 3 * C) and w_proj.shape == (C, C)

    nc, rt = _get_rt()
    in_maps = _host_prep(x, w_qkv, w_proj)
    if rt is not None:
        results = rt.run(in_maps)
    else:
        results = run_bass_kernel_spmd(nc, in_maps,
                                       core_ids=list(range(N_CORES))).results

    out = np.zeros((B, T, C), dtype=np.float32)
    for c in range(N_CORES):
        b = c // GROUPS
        out[b] += results[c]["outT"].astype(np.float32).T
    return out


# revision 30
# speedup vs baseline: 1.4533x; 1.0385x over previous
"""Causal self-attention with RoPE (B=2, T=2048, C=2048, 16 heads) on 8 TRN2
NeuronCores.

Sharding: data-parallel over batch x tensor-parallel over heads.
Core c handles batch c//4 and heads 4*(c%4) .. 4*(c%4)+4. Each core computes
its heads' q/k/v projections, RoPE, causal attention, and a partial output
projection over its heads' channels; the host sums the 4 partial projections
per batch (the tensor-parallel reduce) and stacks the batches.

Per-core program (fp8 DoubleRow projections, fp16 attention, fp32 accum):
  A2: v[T, hd*4] = x.T @ wv as fp8e4m3 hi/lo 3-term (xh*wh + xh*wl + xl*wh)
      DoubleRow matmuls over 256-row contraction slabs; weights host-scaled
      x64, descale folded into the PSUM->SBUF copy (ACT, scale=1/64).
  A1: qT/kT[hd, T] = wqk.T @ x, same fp8 3-term DoubleRow; RoPE applied from
      the fp16 copy with fp16 tables (sign of sin folded into the table):
      dst = qraw*cos2 + rothalf(qraw)*sin2, all DVE.
  B:  per (q-tile, head): scoresT = k.T @ q in fp16 -> psS pairs; one exp per
      [128,1024] pair (ACT); diagonal masks post-exp (DVE); att accumulated
      into acc (DVE) for the denominator, finished by a single ones.T@acc
      matmul; y accumulates in PSUM; y/denom normalized and split into
      fp8 hi/lo (DVE) for the projection.
  C:  outT[C, T] partial = wp.T @ y8 as fp8 3-term DoubleRow, interleaved
      with the next q-tile's attention; psO evacuated by ACT/DVE copies
      (descale 1/64) and DMA'd out.
"""
import os
import numpy as np
from contextlib import ExitStack

os.environ.setdefault("JAX_COMPILATION_CACHE_DIR", "/tmp/jax_comp_cache")

import concourse.bass as bass
import concourse.tile as tile
from concourse import bacc, mybir
from concourse.bass_utils import run_bass_kernel_spmd
from concourse._compat import axon_active

FP16 = mybir.dt.float16
FP32 = mybir.dt.float32
FP8 = mybir.dt.float8e4

B, T, C, NH = 2, 2048, 2048, 16
HD = C // NH
N_CORES = 8
GROUPS = N_CORES // B
HPC = NH // GROUPS
QN = 512
AN = 1024
WS = 64.0                 # host weight scale (keeps fp8 out of subnormals)
SLAB = 256                # DoubleRow contraction slab
NSLAB = C // SLAB


def _build_nc(T=2048, C=2048, HPC=4, n_cores=8, qn=512, reps=1, an=AN):
    HD = 128
    QK_COLS = 2 * HPC * HD   # q+k columns per core (1024)
    V_COLS = HPC * HD        # v columns per core (512)
    NQT = T // qn            # q tiles in attention
    NKC = T // 128           # k chunks
    NSL = C // 256           # contraction slabs for DoubleRow
    scale = 1.0 / np.sqrt(np.float32(HD))
    DR = mybir.MatmulPerfMode.DoubleRow

    nc = bacc.Bacc("TRN2", target_bir_lowering=False, debug=False,
                   num_devices=n_cores)
    x8h_ap = nc.dram_tensor("x8h", (C, T), FP8, kind="ExternalInput").ap()
    x8l_ap = nc.dram_tensor("x8l", (C, T), FP8, kind="ExternalInput").ap()
    wqkh_ap = nc.dram_tensor("wqkh", (C, QK_COLS), FP8, kind="ExternalInput").ap()
    wqkl_ap = nc.dram_tensor("wqkl", (C, QK_COLS), FP8, kind="ExternalInput").ap()
    wvh_ap = nc.dram_tensor("wvh", (C, V_COLS), FP8, kind="ExternalInput").ap()
    wvl_ap = nc.dram_tensor("wvl", (C, V_COLS), FP8, kind="ExternalInput").ap()
    wph_ap = nc.dram_tensor("wph", (128, 4 * C), FP8, kind="ExternalInput").ap()
    wpl_ap = nc.dram_tensor("wpl", (128, 4 * C), FP8, kind="ExternalInput").ap()
    cos_ap = nc.dram_tensor("cos2", (128, T), FP16, kind="ExternalInput").ap()
    sin_ap = nc.dram_tensor("sin2", (128, T), FP16, kind="ExternalInput").ap()
    mask_ap = nc.dram_tensor("masks", (128, 4 * qn), FP16, kind="ExternalInput").ap()
    out_ap = nc.dram_tensor("outT", (C, T), FP16, kind="ExternalOutput").ap()

    x8h_r = x8h_ap.rearrange("(s p two) t -> p s two t", p=128, two=2)
    x8l_r = x8l_ap.rearrange("(s p two) t -> p s two t", p=128, two=2)
    wqkh_r = wqkh_ap.rearrange("(s p two) n -> p s two n", p=128, two=2)
    wqkl_r = wqkl_ap.rearrange("(s p two) n -> p s two n", p=128, two=2)
    wvh_r = wvh_ap.rearrange("(s p two) n -> p s two n", p=128, two=2)
    wvl_r = wvl_ap.rearrange("(s p two) n -> p s two n", p=128, two=2)

    with tile.TileContext(nc) as tc:
      for rep in range(reps):
        R = f"r{rep}_"
        with ExitStack() as top:
            xt_pool = top.enter_context(tc.tile_pool(name=R + "xt", bufs=1))
            qk_pool = top.enter_context(tc.tile_pool(name=R + "qk", bufs=1))
            v_pool = top.enter_context(tc.tile_pool(name=R + "v", bufs=1))
            tab_pool = top.enter_context(tc.tile_pool(name=R + "tab", bufs=1))

            xh = xt_pool.tile([128, NSL, 2, T], FP8)
            xl = xt_pool.tile([128, NSL, 2, T], FP8)
            qk_sb = qk_pool.tile([128, 2 * HPC, T], FP16)  # cols 0..3 q, 4..7 k
            v_sb = v_pool.tile([128, NKC, V_COLS], FP16)   # [t_lo, t_chunk, vcol]
            cos_t = tab_pool.tile([128, T], FP16)
            sin_t = tab_pool.tile([128, T], FP16)
            msk = tab_pool.tile([128, 4, qn], FP16)
            ones_t = tab_pool.tile([128, 1], FP16)
            wph = tab_pool.tile([128, 2, 2, C], FP8)
            wpl = tab_pool.tile([128, 2, 2, C], FP8)

            warm = tab_pool.tile([1, 1], FP32)
            nc.vector.memset(warm[:], 0.0)
            nc.vector.memset(ones_t[:], 1.0)
            warm2 = tab_pool.tile([1, 1], FP32)
            nc.scalar.activation(warm2[:], warm[:],
                                 mybir.ActivationFunctionType.Exp)
            wsrc = tab_pool.tile([128, 16], FP16)
            nc.vector.memset(wsrc[:], 0.0)
            qraw0 = tab_pool.tile([128, 2, QK_COLS // 2], FP16)

            # ---- Phase A2: v projection (x DMAs pipeline into group 0) ----
            TH = T // 2
            wqk_pool = top.enter_context(tc.tile_pool(name=R + "wqk", bufs=1))
            wqh = wqk_pool.tile([128, NSL, 2, QK_COLS], FP8)
            wql = wqk_pool.tile([128, NSL, 2, QK_COLS], FP8)
            with ExitStack() as sV:
                wv_pool = sV.enter_context(tc.tile_pool(name=R + "wv", bufs=1))
                psV_pool = sV.enter_context(
                    tc.tile_pool(name=R + "psV", bufs=8, space="PSUM"))
                # climb the PE pstate ramp while the first x slabs stream in
                pw = psV_pool.tile([128, V_COLS], FP32, tag="psV",
                                   name=R + "pwarm")
                for wi in range(220):
                    nc.tensor.matmul(pw[0:16, 0:16], wsrc[:, 0:16], wsrc[:],
                                     start=(wi == 0), stop=(wi == 219))
                wvh = wv_pool.tile([128, NSL, 2, V_COLS], FP8)
                wvl = wv_pool.tile([128, NSL, 2, V_COLS], FP8)
                # DMA order = consumption order: group 0 needs x T0 + wv
                # slab-by-slab; then group 1's x T1 interleaved with wqk
                # (A1 g0 starts right after A2); tables/masks next (RoPE and
                # the first attention units); wp last (first proj ~30us in).
                for s in range(NSL):
                    nc.sync.dma_start(xh[:, s, :, 0:TH], x8h_r[:, s, :, 0:TH])
                    nc.sync.dma_start(xl[:, s, :, 0:TH], x8l_r[:, s, :, 0:TH])
                    nc.sync.dma_start(wvh[:, s, :, :], wvh_r[:, s, :, :])
                    nc.sync.dma_start(wvl[:, s, :, :], wvl_r[:, s, :, :])
                for s in range(NSL):
                    nc.sync.dma_start(xh[:, s, :, TH:T], x8h_r[:, s, :, TH:T])
                    nc.sync.dma_start(xl[:, s, :, TH:T], x8l_r[:, s, :, TH:T])
                for s in range(NSL):
                    nc.sync.dma_start(wqh[:, s, :, :], wqkh_r[:, s, :, :])
                    nc.sync.dma_start(wql[:, s, :, :], wqkl_r[:, s, :, :])
                nc.sync.dma_start(cos_t[:], cos_ap[:])
                nc.sync.dma_start(sin_t[:], sin_ap[:])
                nc.sync.dma_start(msk[:], mask_ap.rearrange("p (j n) -> p j n", n=qn))
                nc.sync.dma_start(wph[:], wph_ap.rearrange("p (s i c) -> p s i c", s=2, i=2))
                nc.sync.dma_start(wpl[:], wpl_ap.rearrange("p (s i c) -> p s i c", s=2, i=2))
                for grp0 in range(0, NKC, 8):
                    psvs = []
                    for ts in range(grp0, grp0 + 8):
                        psv = psV_pool.tile([128, V_COLS], FP32, tag="psV",
                                            name=f"{R}psV_{ts}")
                        psvs.append(psv)
                    for s in range(NSL):
                        for gi, ts in enumerate(range(grp0, grp0 + 8)):
                            for term, (xs, ws) in enumerate(
                                    ((xh, wvh), (xh, wvl), (xl, wvh))):  # noqa
                                nc.tensor.matmul(
                                    psvs[gi][:],
                                    xs[:, s, :, ts * 128:(ts + 1) * 128],
                                    ws[:, s, :, :],
                                    start=(s == 0 and term == 0),
                                    stop=(s == NSL - 1 and term == 2),
                                    perf_mode=DR)
                            if s == NSL - 1:
                                # alternate engines so the trailing copies
                                # drain in parallel (PSUM handover gate)
                                if ts % 2 == 0:
                                    nc.scalar.activation(
                                        v_sb[:, ts, :], psvs[gi][:],
                                        mybir.ActivationFunctionType.Copy,
                                        scale=float(1.0 / WS))
                                else:
                                    nc.vector.tensor_scalar_mul(
                                        v_sb[:, ts, :], psvs[gi][:],
                                        float(1.0 / WS))

                # A1 g0 head-0 pair rides the psV ring so PE stays busy
                # through the PSUM pool handover
                pva = psV_pool.tile([128, V_COLS], FP32, tag="psV",
                                    name=R + "psA0k")
                pvb = psV_pool.tile([128, V_COLS], FP32, tag="psV",
                                    name=R + "psA0q")
                for s in range(NSL):
                    for term, (ws, xs) in enumerate(
                            ((wqh, xh), (wqh, xl), (wql, xh))):
                        nc.tensor.matmul(
                            pva[:], ws[:, s, :, HPC * HD:(HPC + 1) * HD],
                            xs[:, s, :, 0:qn],
                            start=(s == 0 and term == 0),
                            stop=(s == NSL - 1 and term == 2),
                            perf_mode=DR)
                for s in range(NSL):
                    for term, (ws, xs) in enumerate(
                            ((wqh, xh), (wqh, xl), (wql, xh))):
                        nc.tensor.matmul(
                            pvb[:], ws[:, s, :, 0:HD],
                            xs[:, s, :, 0:qn],
                            start=(s == 0 and term == 0),
                            stop=(s == NSL - 1 and term == 2),
                            perf_mode=DR)
                nc.scalar.activation(
                    qraw0[:, 0, :], pva[:],
                    mybir.ActivationFunctionType.Copy, scale=float(1.0 / WS))
                nc.scalar.activation(
                    qraw0[:, 1, :], pvb[:],
                    mybir.ActivationFunctionType.Copy, scale=float(1.0 / WS))

            # ---- Merged A1 (q/k proj + RoPE) and B/C (attention + proj) ----
            with ExitStack() as sM:
                tmp_pool = sM.enter_context(tc.tile_pool(name=R + "tmp", bufs=2))
                att_pool = sM.enter_context(tc.tile_pool(name=R + "att", bufs=3))
                acc_pool = sM.enter_context(tc.tile_pool(name=R + "acc", bufs=2))
                rec_pool = sM.enter_context(tc.tile_pool(name=R + "rec", bufs=2))
                y8_pool = sM.enter_context(tc.tile_pool(name=R + "y8", bufs=2))
                o_pool = sM.enter_context(tc.tile_pool(name=R + "o", bufs=3))
                psS_pool = sM.enter_context(
                    tc.tile_pool(name=R + "psS", bufs=2, space="PSUM"))
                psY_pool = sM.enter_context(
                    tc.tile_pool(name=R + "psY", bufs=2, space="PSUM"))
                psO_pool = sM.enter_context(
                    tc.tile_pool(name=R + "psO", bufs=2, space="PSUM"))



                NCT = C // 128
                diag_per_qt = qn // 128

                def emit_a1_tile(col, tq, ps_half):
                    """24 DoubleRow matmuls into one psS half, then RoPE."""
                    for s in range(NSL):
                        for term, (ws, xs) in enumerate(
                                ((wqh, xh), (wqh, xl), (wql, xh))):
                            nc.tensor.matmul(
                                ps_half,
                                ws[:, s, :, col * HD:(col + 1) * HD],
                                xs[:, s, :, tq * qn:(tq + 1) * qn],
                                start=(s == 0 and term == 0),
                                stop=(s == NSL - 1 and term == 2),
                                perf_mode=DR)

                def emit_rope(col, tq, qraw):
                    ts0, ts1 = tq * qn, (tq + 1) * qn
                    tcos = tmp_pool.tile([128, qn], FP16, tag="tcos", bufs=1,
                                         name=f"{R}tcos_{col}_{tq}")
                    tsin = tmp_pool.tile([128, qn], FP16, tag="tsin", bufs=1,
                                         name=f"{R}tsin_{col}_{tq}")
                    nc.vector.tensor_mul(tcos[:], qraw[:], cos_t[:, ts0:ts1])
                    nc.vector.tensor_mul(tsin[0:64, :], qraw[64:128, :],
                                         sin_t[64:128, ts0:ts1])
                    nc.vector.tensor_mul(tsin[64:128, :], qraw[0:64, :],
                                         sin_t[0:64, ts0:ts1])
                    nc.vector.tensor_add(qk_sb[:, col, ts0:ts1],
                                         tcos[:], tsin[:])

                def pair_qoff(qt, j):
                    # diag pair whose two chunks mask off q < 256 entirely
                    return 256 if (2 * j - diag_per_qt * qt) >= 2 else 0

                def emit_scores(qt, h, j, ps):
                    qoff = pair_qoff(qt, j)
                    for half in range(2):
                        kc = 2 * j + half
                        nc.tensor.matmul(
                            ps[:, half, qoff:],
                            qk_sb[:, HPC + h, kc * 128:(kc + 1) * 128],
                            qk_sb[:, h, qt * qn + qoff:(qt + 1) * qn],
                            start=True, stop=True)

                def emit_unit_post(qt, h, j, np2, ps, att, acc, psY):
                    qoff = pair_qoff(qt, j)
                    nc.scalar.activation(
                        att[:, :, qoff:], ps[:, :, qoff:],
                        mybir.ActivationFunctionType.Exp, scale=float(scale))
                    for half in range(2):
                        kc = 2 * j + half
                        jj = kc - diag_per_qt * qt
                        if jj >= 0:
                            nc.vector.tensor_mul(att[:, half, qoff:],
                                                 att[:, half, qoff:],
                                                 msk[:, jj, qoff:])
                    if j == 0:
                        nc.vector.tensor_copy(acc[:, :, :], att[:, :, :])
                    else:
                        nc.vector.tensor_add(acc[:, :, qoff:], acc[:, :, qoff:],
                                             att[:, :, qoff:])
                    for half in range(2):
                        kc = 2 * j + half
                        nc.tensor.matmul(
                            psY[:, qoff:], v_sb[:, kc, h * HD:(h + 1) * HD],
                            att[:, half, qoff:],
                            start=(kc == 0), stop=(kc == 2 * np2 - 1))

                def emit_norm(qt, h, acc, psY):
                    accf = acc_pool.tile([128, qn], FP16, tag="accf", bufs=1,
                                         name=f"{R}accf_{h}_{qt}")
                    nc.vector.tensor_add(accf[:], acc[:, 0, :], acc[:, 1, :])
                    psD = psO_pool.tile([128, qn], FP32, tag="psO",
                                        name=f"{R}psD_{h}_{qt}")
                    nc.tensor.matmul(psD[0:1, :], ones_t[:], accf[:],
                                     start=True, stop=True)
                    rec = rec_pool.tile([1, qn], FP32, tag="rec", bufs=1,
                                        name=f"{R}rec_{h}_{qt}")
                    nc.vector.reciprocal(rec[:], psD[0:1, :])
                    recb = rec_pool.tile([128, qn], FP32, tag="recb",
                                         bufs=1, name=f"{R}recb_{h}_{qt}")
                    nc.gpsimd.partition_broadcast(recb[:], rec[:])
                    yt = rec_pool.tile([128, qn], FP32, tag="yt", bufs=1,
                                       name=f"{R}yt_{h}_{qt}")
                    nc.vector.tensor_mul(yt[:], psY[:], recb[:])
                    y8h, y8l = y8_of_qt[qt]
                    nc.vector.tensor_copy(y8h[:, h, :], yt[:])
                    nc.vector.tensor_sub(y8l[:, h, :], yt[:], y8h[:, h, :])

                def emit_proj_ct(qt, ct, evac_dve, tag_rot=False):
                    y8h, y8l = y8_of_qt[qt]
                    if tag_rot and ct % 2 == 1:
                        psw = psS_pool.tile([128, 2, qn], FP32, tag="psS",
                                            name=f"{R}psOS_{ct}_{qt}")
                        pso = psw[:, 0, :]
                    elif tag_rot and ct % 4 == 2:
                        pso = psY_pool.tile([128, qn], FP32, tag="psY",
                                            name=f"{R}psOY_{ct}_{qt}")
                    else:
                        pso = psO_pool.tile([128, 512], FP32, tag="psO",
                                            name=f"{R}psO_{ct}_{qt}")
                    n = 0
                    for s in range(2):
                        for (wp_, y8_) in ((wph, y8h), (wph, y8l), (wpl, y8h)):
                            nc.tensor.matmul(
                                pso[:], wp_[:, s, :, ct * 128:(ct + 1) * 128],
                                y8_[:, 2 * s:2 * s + 2, :],
                                start=(n == 0), stop=(n == 5),
                                perf_mode=DR)
                            n += 1
                    o_t = o_pool.tile([128, 512], FP16, tag="o",
                                      name=f"{R}o_{ct}_{qt}")
                    if evac_dve:
                        nc.vector.tensor_scalar_mul(o_t[:], pso[:],
                                                    float(1.0 / WS))
                    else:
                        nc.scalar.activation(o_t[:], pso[:],
                                             mybir.ActivationFunctionType.Copy,
                                             scale=float(1.0 / WS))
                    nc.sync.dma_start(
                        out_ap[ct * 128:(ct + 1) * 128, qt * qn:(qt + 1) * qn],
                        o_t[:])

                # global pipeline: per tq group, A1 pair-tiles and proj of
                # qt-1 interleaved into qt's units via a side-work queue.
                prev = [None, None]   # ps, meta

                def flush_prev(i_slot):
                    if prev[0] is None:
                        return
                    pqt, ph, pj, pnp2 = prev[1]
                    st = state[(pqt, ph)]
                    att = att_pool.tile([128, 2, qn], FP16, tag="att",
                                        name=f"{R}att_{ph}_{pqt}_{pj}")
                    emit_unit_post(pqt, ph, pj, pnp2, prev[0], att,
                                   st["acc"], st["psY"])
                    if pj == pnp2 - 1:
                        emit_norm(pqt, ph, st["acc"], st["psY"])
                    prev[0] = None

                state = {}
                y8_of_qt = {}
                side_q = []   # [due_slot, fn]
                slot = [0]

                def emit_a1_pair(g, h):
                    ps = psS_pool.tile([128, 2, qn], FP32, tag="psS",
                                       name=f"{R}psA_{g}_{h}")
                    emit_a1_tile(HPC + h, g, ps[:, 0, :])   # k col
                    emit_a1_tile(h, g, ps[:, 1, :])         # q col
                    qraw = tmp_pool.tile([128, 2, qn], FP16, tag="qraw",
                                         bufs=1, name=f"{R}qraw_{g}_{h}")
                    nc.scalar.activation(
                        qraw[:, :, :], ps[:, :, :],
                        mybir.ActivationFunctionType.Copy,
                        scale=float(1.0 / WS))
                    emit_rope(HPC + h, g, qraw[:, 0, :])
                    emit_rope(h, g, qraw[:, 1, :])

                # A1 group 0: pair h0's matmuls ran in the A2 scope;
                # rope it here, then the remaining pairs
                emit_rope(HPC + 0, 0, qraw0[:, 0, :])
                emit_rope(0, 0, qraw0[:, 1, :])
                for h in range(1, HPC):
                    emit_a1_pair(0, h)

                for g in range(NQT):
                    np2 = diag_per_qt * (g + 1) // 2
                    U = HPC * np2
                    s0 = slot[0]
                    # pre-queue side work for this stretch of units
                    if g >= 1:
                        for ct in range(NCT):
                            side_q.append(
                                [s0 + 2 + ct // 4,
                                 (lambda pqt, pct, pdve:
                                  lambda: emit_proj_ct(pqt, pct, pdve))(
                                     g - 1, ct, ct % 2 == 0)])
                    side_q.sort(key=lambda it: it[0])
                    if g >= 1:
                        for h in range(HPC):
                            emit_a1_pair(g, h)
                    for h in range(HPC):
                        for j in range(np2):
                            if j == 0 and h == 0:
                                y8_of_qt[g] = (
                                    y8_pool.tile([128, HPC, qn], FP8,
                                                 tag="y8h", name=f"{R}y8h_{g}"),
                                    y8_pool.tile([128, HPC, qn], FP8,
                                                 tag="y8l", name=f"{R}y8l_{g}"),
                                )
                            if j == 0:
                                state[(g, h)] = {
                                    "acc": acc_pool.tile(
                                        [128, 2, qn], FP16, tag="acc",
                                        name=f"{R}acc_{h}_{g}"),
                                    "psY": psY_pool.tile(
                                        [128, qn], FP32, tag="psY",
                                        name=f"{R}psY_{h}_{g}"),
                                }
                            ps = psS_pool.tile([128, 2, qn], FP32, tag="psS",
                                               name=f"{R}psS_{h}_{g}_{j}")
                            emit_scores(g, h, j, ps)
                            flush_prev(slot[0])
                            prev[0] = ps
                            prev[1] = (g, h, j, np2)
                            while side_q and side_q[0][0] <= slot[0]:
                                side_q.pop(0)[1]()
                            slot[0] += 1
                    # group end: ensure last unit's consumers are emitted
                    # before the next allocations recycle its psS slot
                    flush_prev(slot[0])
                    while side_q:
                        side_q.pop(0)[1]()
                # final projection for the last q-tile; alternate PSUM tags
                # (psS slots are idle by now) for a deeper evac pipeline
                for ct in range(NCT):
                    emit_proj_ct(NQT - 1, ct, ct % 2 == 0, tag_rot=True)
    nc.compile()
    return nc


_CACHE = {}


def _rope_tables_np(t_len, hd):
    inv_freq = 1.0 / (10000.0 ** (np.arange(0, hd, 2, dtype=np.float32) / hd))
    t = np.arange(t_len, dtype=np.float32)
    freqs = np.outer(t, inv_freq)
    emb = np.concatenate([freqs, freqs], axis=-1)
    return np.cos(emb)[:, ::2].astype(np.float32), np.sin(emb)[:, ::2].astype(np.float32)


def _static_arrays():
    if "static" not in _CACHE:
        cos_, sin_ = _rope_tables_np(T, HD)
        cosT = np.ascontiguousarray(cos_.T)   # (64, T)
        sinT = np.ascontiguousarray(sin_.T)
        # sin table halves are indexed by the *source* partition of the
        # rotate-half read: rows 64..127 hold -sin (multiplies x2 into the
        # low half), rows 0..63 hold +sin (multiplies x1 into the high half).
        cos2 = np.concatenate([cosT, cosT], axis=0).astype(np.float16)
        sin2 = np.concatenate([sinT, -sinT], axis=0).astype(np.float16)
        perm = np.concatenate([np.arange(0, HD, 2), np.arange(1, HD, 2)])
        p = np.arange(128)[:, None]
        f = np.arange(QN)[None, :]
        masks = np.concatenate(
            [(p <= (f - 128 * j)).astype(np.float16) for j in range(QN // 128)],
            axis=1)
        _CACHE["static"] = (cos2, sin2, perm, masks)
    return _CACHE["static"]


def _q8(a, np8):
    hi = a.astype(np8)
    lo = (a - hi.astype(np.float32)).astype(np8)
    return hi, lo


def _host_prep(x, w_qkv, w_proj):
    cos2, sin2, perm, masks = _static_arrays()
    np8 = mybir.dt.np(FP8)

    wq = w_qkv[:, 0 * C:1 * C]
    wk = w_qkv[:, 1 * C:2 * C]
    wv = w_qkv[:, 2 * C:3 * C]

    in_maps = []
    xq = {}
    for b in range(B):
        xT = np.ascontiguousarray(x[b].T)
        xq[b] = _q8(xT, np8)
    for c in range(N_CORES):
        b = c // GROUPS
        hg = c % GROUPS
        hs = slice(hg * HPC * HD, (hg + 1) * HPC * HD)
        wq_c = wq[:, hs].reshape(C, HPC, HD)[:, :, perm].reshape(C, HPC * HD)
        wk_c = wk[:, hs].reshape(C, HPC, HD)[:, :, perm].reshape(C, HPC * HD)
        wqk_c = np.concatenate([wq_c, wk_c], axis=1) * WS
        wqkh, wqkl = _q8(wqk_c, np8)
        wvh, wvl = _q8(wv[:, hs] * WS, np8)
        # wp arranged [d, s, i, c]: row d holds heads (2s+i) of this group
        wp_c = (w_proj[hs, :] * WS).reshape(2, 2, 128, C).transpose(2, 0, 1, 3)
        wph, wpl = _q8(np.ascontiguousarray(wp_c.reshape(128, 4 * C)), np8)
        x8h, x8l = xq[b]
        in_maps.append({
            "x8h": x8h, "x8l": x8l,
            "wqkh": wqkh, "wqkl": wqkl,
            "wvh": wvh, "wvl": wvl,
            "wph": wph, "wpl": wpl,
            "cos2": cos2, "sin2": sin2,
            "masks": masks,
        })
    return in_maps


class _PjrtRunner:
    """Caches the jitted shard_map callable so repeat kernel() calls skip
    retracing. Mirrors concourse.bass2jax.run_bass_via_pjrt."""

    def __init__(self, nc):
        import jax
        from jax.sharding import Mesh, PartitionSpec, NamedSharding
        from jax.experimental.shard_map import shard_map
        from concourse.bass2jax import (
            _bass_exec_p, install_neuronx_cc_hook, partition_id_tensor)

        install_neuronx_cc_hook()
        self.jax = jax
        partition_name = nc.partition_id_tensor.name if nc.partition_id_tensor else None
        in_names, out_names, out_avals = [], [], []
        for alloc in nc.m.functions[0].allocations:
            if not isinstance(alloc, mybir.MemoryLocationSet):
                continue
            name = alloc.memorylocations[0].name
            if alloc.kind == "ExternalInput":
                if name != partition_name:
                    in_names.append(name)
            elif alloc.kind == "ExternalOutput":
                out_names.append(name)
                out_avals.append(jax.core.ShapedArray(
                    tuple(alloc.tensor_shape), mybir.dt.np(alloc.dtype)))
        self.in_names, self.out_names, self.out_avals = in_names, out_names, out_avals
        n_params = len(in_names)
        n_outs = len(out_avals)
        bind_names = tuple(in_names + out_names +
                           ([partition_name] if partition_name else []))
        donate = tuple(range(n_params, n_params + n_outs))

        def _body(*args):
            operands = list(args)
            if partition_name:
                operands.append(partition_id_tensor())
            outs = _bass_exec_p.bind(
                *operands,
                out_avals=tuple(out_avals),
                in_names=bind_names,
                out_names=tuple(out_names),
                lowering_input_output_aliases=(),
                sim_require_finite=True,
                sim_require_nnan=True,
                nc=nc,
            )
            return tuple(outs)

        devices = jax.devices()[:N_CORES]
        self.mesh = Mesh(np.asarray(devices), ("core",))
        self.sharding = NamedSharding(self.mesh, PartitionSpec("core"))
        in_specs = (PartitionSpec("core"),) * (n_params + n_outs)
        out_specs = (PartitionSpec("core"),) * len(out_names)
        self.fn = jax.jit(
            shard_map(_body, mesh=self.mesh, in_specs=in_specs,
                      out_specs=out_specs, check_rep=False),
            donate_argnums=donate,
        )

    def run(self, in_maps):
        jax = self.jax
        concat = [
            np.concatenate([np.asarray(m[name]) for m in in_maps], axis=0)
            for name in self.in_names
        ]
        dev = [jax.device_put(c, self.sharding) for c in concat]
        zeros = [
            jax.device_put(
                np.zeros((N_CORES * a.shape[0], *a.shape[1:]), a.dtype),
                self.sharding)
            for a in self.out_avals
        ]
        outs = self.fn(*dev, *zeros)
        jax.block_until_ready(outs)
        res = []
        for c in range(N_CORES):
            d = {}
            for i, name in enumerate(self.out_names):
                a = np.asarray(outs[i])
                d[name] = a.reshape(N_CORES, *self.out_avals[i].shape)[c]
            res.append(d)
        return res


def _get_rt():
    if "rt" not in _CACHE:
        nc = _build_nc(T=T, C=C, HPC=HPC, n_cores=N_CORES, qn=QN, reps=1, an=AN)
        _CACHE["nc"] = nc
        _CACHE["rt"] = _PjrtRunner(nc) if axon_active() else None
    return _CACHE.get("nc"), _CACHE.get("rt")


def kernel(x, w_qkv, w_proj, n_head):
    assert int(n_head) == NH
    x = np.asarray(x, dtype=np.float32)
    w_qkv = np.asarray(w_qkv, dtype=np.float32)
    w_proj = np.asarray(w_proj, dtype=np.float32)
    assert x.shape == (B, T, C) and w_qkv.shape == (C, 3 * C) and w_proj.shape == (C, C)

    nc, rt = _get_rt()
    in_maps = _host_prep(x, w_qkv, w_proj)
    if rt is not None:
        results = rt.run(in_maps)
    else:
        results = run_bass_kernel_spmd(nc, in_maps,
                                       core_ids=list(range(N_CORES))).results

    out = np.zeros((B, T, C), dtype=np.float32)
    for c in range(N_CORES):
        b = c // GROUPS
        out[b] += results[c]["outT"].astype(np.float32).T
    return out


# revision 39
# speedup vs baseline: 1.4759x; 1.0155x over previous
"""Causal self-attention with RoPE (B=2, T=2048, C=2048, 16 heads) on 8 TRN2
NeuronCores.

Sharding: data-parallel over batch x tensor-parallel over heads.
Core c handles batch c//4 and heads 4*(c%4) .. 4*(c%4)+4. Each core computes
its heads' q/k/v projections, RoPE, causal attention, and a partial output
projection over its heads' channels; the host sums the 4 partial projections
per batch (the tensor-parallel reduce) and stacks the batches.

Per-core program (fp8 DoubleRow projections, fp16 attention, fp32 accum):
  A2: v[T, hd*4] = x.T @ wv as fp8e4m3 hi/lo 3-term (xh*wh + xh*wl + xl*wh)
      DoubleRow matmuls over 256-row contraction slabs; weights host-scaled
      x64, descale folded into the PSUM->SBUF copy (ACT, scale=1/64).
  A1: qT/kT[hd, T] = wqk.T @ x, same fp8 3-term DoubleRow; RoPE applied from
      the fp16 copy with fp16 tables (sign of sin folded into the table):
      dst = qraw*cos2 + rothalf(qraw)*sin2, all DVE.
  B:  per (q-tile, head): scoresT = k.T @ q in fp16 -> psS pairs; one exp per
      [128,1024] pair (ACT); diagonal masks post-exp (DVE); att accumulated
      into acc (DVE) for the denominator, finished by a single ones.T@acc
      matmul; y accumulates in PSUM; y/denom normalized and split into
      fp8 hi/lo (DVE) for the projection.
  C:  outT[C, T] partial = wp.T @ y8 as fp8 3-term DoubleRow, interleaved
      with the next q-tile's attention; psO evacuated by ACT/DVE copies
      (descale 1/64) and DMA'd out.
"""
import os
import numpy as np
from contextlib import ExitStack

os.environ.setdefault("JAX_COMPILATION_CACHE_DIR", "/tmp/jax_comp_cache")

import concourse.bass as bass
import concourse.tile as tile
from concourse import bacc, mybir
from concourse.bass_utils import run_bass_kernel_spmd
from concourse._compat import axon_active

FP16 = mybir.dt.float16
FP32 = mybir.dt.float32
FP8 = mybir.dt.float8e4

B, T, C, NH = 2, 2048, 2048, 16
HD = C // NH
N_CORES = 8
GROUPS = N_CORES // B
HPC = NH // GROUPS
QN = 512
AN = 1024
WS = 64.0                 # host weight scale (keeps fp8 out of subnormals)
SLAB = 256                # DoubleRow contraction slab
NSLAB = C // SLAB


def _build_nc(T=2048, C=2048, HPC=4, n_cores=8, qn=512, reps=1, an=AN):
    HD = 128
    QK_COLS = 2 * HPC * HD   # q+k columns per core (1024)
    V_COLS = HPC * HD        # v columns per core (512)
    NQT = T // qn            # q tiles in attention
    NKC = T // 128           # k chunks
    NSL = C // 256           # contraction slabs for DoubleRow
    scale = 1.0 / np.sqrt(np.float32(HD))
    DR = mybir.MatmulPerfMode.DoubleRow

    nc = bacc.Bacc("TRN2", target_bir_lowering=False, debug=False,
                   num_devices=n_cores)
    x8h_ap = nc.dram_tensor("x8h", (C, T), FP8, kind="ExternalInput").ap()
    x8l_ap = nc.dram_tensor("x8l", (C, T), FP8, kind="ExternalInput").ap()
    wqkh_ap = nc.dram_tensor("wqkh", (C, QK_COLS), FP8, kind="ExternalInput").ap()
    wqkl_ap = nc.dram_tensor("wqkl", (C, QK_COLS), FP8, kind="ExternalInput").ap()
    wvh_ap = nc.dram_tensor("wvh", (C, V_COLS), FP8, kind="ExternalInput").ap()
    wvl_ap = nc.dram_tensor("wvl", (C, V_COLS), FP8, kind="ExternalInput").ap()
    wph_ap = nc.dram_tensor("wph", (128, 4 * C), FP8, kind="ExternalInput").ap()
    wpl_ap = nc.dram_tensor("wpl", (128, 4 * C), FP8, kind="ExternalInput").ap()
    cos_ap = nc.dram_tensor("cos2", (128, T), FP16, kind="ExternalInput").ap()
    sin_ap = nc.dram_tensor("sin2", (128, T), FP16, kind="ExternalInput").ap()
    mask_ap = nc.dram_tensor("masks", (128, 4 * qn), FP16, kind="ExternalInput").ap()
    out_ap = nc.dram_tensor("outT", (C, T), FP16, kind="ExternalOutput").ap()

    x8h_r = x8h_ap.rearrange("(s p two) t -> p s two t", p=128, two=2)
    x8l_r = x8l_ap.rearrange("(s p two) t -> p s two t", p=128, two=2)
    wqkh_r = wqkh_ap.rearrange("(s p two) n -> p s two n", p=128, two=2)
    wqkl_r = wqkl_ap.rearrange("(s p two) n -> p s two n", p=128, two=2)
    wvh_r = wvh_ap.rearrange("(s p two) n -> p s two n", p=128, two=2)
    wvl_r = wvl_ap.rearrange("(s p two) n -> p s two n", p=128, two=2)

    with tile.TileContext(nc) as tc:
      for rep in range(reps):
        R = f"r{rep}_"
        with ExitStack() as top:
            xt_pool = top.enter_context(tc.tile_pool(name=R + "xt", bufs=1))
            qk_pool = top.enter_context(tc.tile_pool(name=R + "qk", bufs=1))
            v_pool = top.enter_context(tc.tile_pool(name=R + "v", bufs=1))
            tab_pool = top.enter_context(tc.tile_pool(name=R + "tab", bufs=1))

            xh = xt_pool.tile([128, NSL, 2, T], FP8)
            xl = xt_pool.tile([128, NSL, 2, T], FP8)
            qk_sb = qk_pool.tile([128, 2 * HPC, T], FP16)  # cols 0..3 q, 4..7 k
            v_sb = v_pool.tile([128, NKC, V_COLS], FP16)   # [t_lo, t_chunk, vcol]
            cos_t = tab_pool.tile([128, T], FP16)
            sin_t = tab_pool.tile([128, T], FP16)
            msk = tab_pool.tile([128, 4, qn], FP16)
            ones_t = tab_pool.tile([128, 1], FP16)
            wph = tab_pool.tile([128, 2, 2, C], FP8)
            wpl = tab_pool.tile([128, 2, 2, C], FP8)

            warm = tab_pool.tile([1, 1], FP32)
            nc.vector.memset(warm[:], 0.0)
            nc.vector.memset(ones_t[:], 1.0)
            warm2 = tab_pool.tile([1, 1], FP32)
            nc.scalar.activation(warm2[:], warm[:],
                                 mybir.ActivationFunctionType.Exp)
            wsrc = tab_pool.tile([128, 16], FP16)
            nc.vector.memset(wsrc[:], 0.0)
            qraw0 = tab_pool.tile([128, 2, QK_COLS // 2], FP16)

            # ---- Phase A2: v projection (x DMAs pipeline into group 0) ----
            TH = T // 2
            wqk_pool = top.enter_context(tc.tile_pool(name=R + "wqk", bufs=1))
            wqh = wqk_pool.tile([128, NSL, 2, QK_COLS], FP8)
            wql = wqk_pool.tile([128, NSL, 2, QK_COLS], FP8)
            with ExitStack() as sV:
                wv_pool = sV.enter_context(tc.tile_pool(name=R + "wv", bufs=1))
                psV_pool = sV.enter_context(
                    tc.tile_pool(name=R + "psV", bufs=8, space="PSUM"))
                # climb the PE pstate ramp while the first x slabs stream in
                pw = psV_pool.tile([128, V_COLS], FP32, tag="psV",
                                   name=R + "pwarm")
                for wi in range(220):
                    nc.tensor.matmul(pw[0:16, 0:16], wsrc[:, 0:16], wsrc[:],
                                     start=(wi == 0), stop=(wi == 219))
                wvh = wv_pool.tile([128, NSL, 2, V_COLS], FP8)
                wvl = wv_pool.tile([128, NSL, 2, V_COLS], FP8)
                # DMA order = consumption order: group 0 needs x T0 + wv
                # slab-by-slab; then group 1's x T1 interleaved with wqk
                # (A1 g0 starts right after A2); tables/masks next (RoPE and
                # the first attention units); wp last (first proj ~30us in).
                for s in range(NSL):
                    nc.sync.dma_start(xh[:, s, :, 0:TH], x8h_r[:, s, :, 0:TH])
                    nc.sync.dma_start(xl[:, s, :, 0:TH], x8l_r[:, s, :, 0:TH])
                    nc.sync.dma_start(wvh[:, s, :, :], wvh_r[:, s, :, :])
                    nc.sync.dma_start(wvl[:, s, :, :], wvl_r[:, s, :, :])
                for s in range(NSL):
                    nc.sync.dma_start(xh[:, s, :, TH:T], x8h_r[:, s, :, TH:T])
                    nc.sync.dma_start(xl[:, s, :, TH:T], x8l_r[:, s, :, TH:T])
                for s in range(NSL):
                    nc.sync.dma_start(wqh[:, s, :, :], wqkh_r[:, s, :, :])
                    nc.sync.dma_start(wql[:, s, :, :], wqkl_r[:, s, :, :])
                nc.sync.dma_start(cos_t[:], cos_ap[:])
                nc.sync.dma_start(sin_t[:], sin_ap[:])
                nc.sync.dma_start(msk[:], mask_ap.rearrange("p (j n) -> p j n", n=qn))
                nc.sync.dma_start(wph[:], wph_ap.rearrange("p (s i c) -> p s i c", s=2, i=2))
                nc.sync.dma_start(wpl[:], wpl_ap.rearrange("p (s i c) -> p s i c", s=2, i=2))
                for grp0 in range(0, NKC, 8):
                    psvs = []
                    for ts in range(grp0, grp0 + 8):
                        psv = psV_pool.tile([128, V_COLS], FP32, tag="psV",
                                            name=f"{R}psV_{ts}")
                        psvs.append(psv)
                    for s in range(NSL):
                        for gi, ts in enumerate(range(grp0, grp0 + 8)):
                            for term, (xs, ws) in enumerate(
                                    ((xh, wvh), (xh, wvl), (xl, wvh))):  # noqa
                                nc.tensor.matmul(
                                    psvs[gi][:],
                                    xs[:, s, :, ts * 128:(ts + 1) * 128],
                                    ws[:, s, :, :],
                                    start=(s == 0 and term == 0),
                                    stop=(s == NSL - 1 and term == 2),
                                    perf_mode=DR)
                            if s == NSL - 1:
                                # alternate engines so the trailing copies
                                # drain in parallel (PSUM handover gate)
                                if ts % 2 == 0:
                                    nc.scalar.activation(
                                        v_sb[:, ts, :], psvs[gi][:],
                                        mybir.ActivationFunctionType.Copy,
                                        scale=float(1.0 / WS))
                                else:
                                    nc.vector.tensor_scalar_mul(
                                        v_sb[:, ts, :], psvs[gi][:],
                                        float(1.0 / WS))

                # A1 g0 head-0 pair rides the psV ring so PE stays busy
                # through the PSUM pool handover
                pva = psV_pool.tile([128, V_COLS], FP32, tag="psV",
                                    name=R + "psA0k")
                pvb = psV_pool.tile([128, V_COLS], FP32, tag="psV",
                                    name=R + "psA0q")
                for s in range(NSL):
                    for term, (ws, xs) in enumerate(
                            ((wqh, xh), (wqh, xl), (wql, xh))):
                        nc.tensor.matmul(
                            pva[:], ws[:, s, :, HPC * HD:(HPC + 1) * HD],
                            xs[:, s, :, 0:qn],
                            start=(s == 0 and term == 0),
                            stop=(s == NSL - 1 and term == 2),
                            perf_mode=DR)
                for s in range(NSL):
                    for term, (ws, xs) in enumerate(
                            ((wqh, xh), (wqh, xl), (wql, xh))):
                        nc.tensor.matmul(
                            pvb[:], ws[:, s, :, 0:HD],
                            xs[:, s, :, 0:qn],
                            start=(s == 0 and term == 0),
                            stop=(s == NSL - 1 and term == 2),
                            perf_mode=DR)
                nc.scalar.activation(
                    qraw0[:, 0, :], pva[:],
                    mybir.ActivationFunctionType.Copy, scale=float(1.0 / WS))
                nc.scalar.activation(
                    qraw0[:, 1, :], pvb[:],
                    mybir.ActivationFunctionType.Copy, scale=float(1.0 / WS))

            # ---- Merged A1 (q/k proj + RoPE) and B/C (attention + proj) ----
            with ExitStack() as sM:
                tmp_pool = sM.enter_context(tc.tile_pool(name=R + "tmp", bufs=2))
                att_pool = sM.enter_context(tc.tile_pool(name=R + "att", bufs=3))
                acc_pool = sM.enter_context(tc.tile_pool(name=R + "acc", bufs=2))
                rec_pool = sM.enter_context(tc.tile_pool(name=R + "rec", bufs=2))
                y8_pool = sM.enter_context(tc.tile_pool(name=R + "y8", bufs=2))
                o_pool = sM.enter_context(tc.tile_pool(name=R + "o", bufs=4))
                psS_pool = sM.enter_context(
                    tc.tile_pool(name=R + "psS", bufs=2, space="PSUM"))
                psY_pool = sM.enter_context(
                    tc.tile_pool(name=R + "psY", bufs=2, space="PSUM"))
                psO_pool = sM.enter_context(
                    tc.tile_pool(name=R + "psO", bufs=2, space="PSUM"))



                NCT = C // 128
                diag_per_qt = qn // 128

                def emit_a1_tile(col, tq, ps_half):
                    """24 DoubleRow matmuls into one psS half, then RoPE."""
                    for s in range(NSL):
                        for term, (ws, xs) in enumerate(
                                ((wqh, xh), (wqh, xl), (wql, xh))):
                            nc.tensor.matmul(
                                ps_half,
                                ws[:, s, :, col * HD:(col + 1) * HD],
                                xs[:, s, :, tq * qn:(tq + 1) * qn],
                                start=(s == 0 and term == 0),
                                stop=(s == NSL - 1 and term == 2),
                                perf_mode=DR)

                def emit_rope(col, tq, qraw):
                    ts0, ts1 = tq * qn, (tq + 1) * qn
                    tcos = tmp_pool.tile([128, qn], FP16, tag="trig", bufs=2,
                                         name=f"{R}tcos_{col}_{tq}")
                    tsin = tmp_pool.tile([128, qn], FP16, tag="trig", bufs=2,
                                         name=f"{R}tsin_{col}_{tq}")
                    nc.vector.tensor_mul(tcos[:], qraw[:], cos_t[:, ts0:ts1])
                    nc.vector.tensor_mul(tsin[0:64, :], qraw[64:128, :],
                                         sin_t[64:128, ts0:ts1])
                    nc.vector.tensor_mul(tsin[64:128, :], qraw[0:64, :],
                                         sin_t[0:64, ts0:ts1])
                    nc.vector.tensor_add(qk_sb[:, col, ts0:ts1],
                                         tcos[:], tsin[:])

                def pair_qoff(qt, j):
                    # diag pair whose two chunks mask off q < 256 entirely
                    return 256 if (2 * j - diag_per_qt * qt) >= 2 else 0

                def emit_scores(qt, h, j, ps):
                    qoff = pair_qoff(qt, j)
                    for half in range(2):
                        kc = 2 * j + half
                        nc.tensor.matmul(
                            ps[:, half, qoff:],
                            qk_sb[:, HPC + h, kc * 128:(kc + 1) * 128],
                            qk_sb[:, h, qt * qn + qoff:(qt + 1) * qn],
                            start=True, stop=True)

                def emit_unit_post(qt, h, j, np2, ps, att, acc, psY):
                    qoff = pair_qoff(qt, j)
                    nc.scalar.activation(
                        att[:, :, qoff:], ps[:, :, qoff:],
                        mybir.ActivationFunctionType.Exp, scale=float(scale))
                    for half in range(2):
                        kc = 2 * j + half
                        jj = kc - diag_per_qt * qt
                        if jj >= 0:
                            nc.vector.tensor_mul(att[:, half, qoff:],
                                                 att[:, half, qoff:],
                                                 msk[:, jj, qoff:])
                    if j == 0:
                        nc.vector.tensor_copy(acc[:, :, :], att[:, :, :])
                    else:
                        nc.vector.tensor_add(acc[:, :, qoff:], acc[:, :, qoff:],
                                             att[:, :, qoff:])
                    for half in range(2):
                        kc = 2 * j + half
                        nc.tensor.matmul(
                            psY[:, qoff:], v_sb[:, kc, h * HD:(h + 1) * HD],
                            att[:, half, qoff:],
                            start=(kc == 0), stop=(kc == 2 * np2 - 1))

                def emit_norm(qt, h, acc, psY):
                    accf = acc_pool.tile([128, qn], FP16, tag="accf", bufs=1,
                                         name=f"{R}accf_{h}_{qt}")
                    nc.vector.tensor_add(accf[:], acc[:, 0, :], acc[:, 1, :])
                    psD = psO_pool.tile([128, qn], FP32, tag="psO",
                                        name=f"{R}psD_{h}_{qt}")
                    nc.tensor.matmul(psD[0:1, :], ones_t[:], accf[:],
                                     start=True, stop=True)
                    rec = rec_pool.tile([1, qn], FP32, tag="rec", bufs=1,
                                        name=f"{R}rec_{h}_{qt}")
                    nc.vector.reciprocal(rec[:], psD[0:1, :])
                    recb = rec_pool.tile([128, qn], FP32, tag="recb",
                                         bufs=1, name=f"{R}recb_{h}_{qt}")
                    nc.gpsimd.partition_broadcast(recb[:], rec[:])
                    yt = rec_pool.tile([128, qn], FP32, tag="yt", bufs=1,
                                       name=f"{R}yt_{h}_{qt}")
                    nc.vector.tensor_mul(yt[:], psY[:], recb[:])
                    y8h, y8l = y8_of_qt[qt]
                    nc.vector.tensor_copy(y8h[:, h, :], yt[:])
                    nc.vector.tensor_sub(y8l[:, h, :], yt[:], y8h[:, h, :])

                def emit_proj_ct(qt, ct, evac_dve, tag_rot=False):
                    y8h, y8l = y8_of_qt[qt]
                    if tag_rot and ct % 2 == 1:
                        psw = psS_pool.tile([128, 2, qn], FP32, tag="psS",
                                            name=f"{R}psOS_{ct}_{qt}")
                        pso = psw[:, 0, :]
                    elif tag_rot and ct % 4 == 2:
                        pso = psY_pool.tile([128, qn], FP32, tag="psY",
                                            name=f"{R}psOY_{ct}_{qt}")
                    else:
                        pso = psO_pool.tile([128, 512], FP32, tag="psO",
                                            name=f"{R}psO_{ct}_{qt}")
                    n = 0
                    for s in range(2):
                        for (wp_, y8_) in ((wph, y8h), (wph, y8l), (wpl, y8h)):
                            nc.tensor.matmul(
                                pso[:], wp_[:, s, :, ct * 128:(ct + 1) * 128],
                                y8_[:, 2 * s:2 * s + 2, :],
                                start=(n == 0), stop=(n == 5),
                                perf_mode=DR)
                            n += 1
                    o_t = o_pool.tile([128, 512], FP16, tag="o",
                                      name=f"{R}o_{ct}_{qt}")
                    if evac_dve:
                        nc.vector.tensor_scalar_mul(o_t[:], pso[:],
                                                    float(1.0 / WS))
                    else:
                        nc.scalar.activation(o_t[:], pso[:],
                                             mybir.ActivationFunctionType.Copy,
                                             scale=float(1.0 / WS))
                    nc.sync.dma_start(
                        out_ap[ct * 128:(ct + 1) * 128, qt * qn:(qt + 1) * qn],
                        o_t[:])

                # global pipeline: per tq group, A1 pair-tiles and proj of
                # qt-1 interleaved into qt's units via a side-work queue.
                prev = [None, None]   # ps, meta

                def flush_prev(i_slot):
                    if prev[0] is None:
                        return
                    pqt, ph, pj, pnp2 = prev[1]
                    st = state[(pqt, ph)]
                    att = att_pool.tile([128, 2, qn], FP16, tag="att",
                                        name=f"{R}att_{ph}_{pqt}_{pj}")
                    emit_unit_post(pqt, ph, pj, pnp2, prev[0], att,
                                   st["acc"], st["psY"])
                    if pj == pnp2 - 1:
                        emit_norm(pqt, ph, st["acc"], st["psY"])
                    prev[0] = None

                state = {}
                y8_of_qt = {}
                side_q = []   # [due_slot, fn]
                slot = [0]

                def emit_a1_pair(g, h):
                    ps = psS_pool.tile([128, 2, qn], FP32, tag="psS",
                                       name=f"{R}psA_{g}_{h}")
                    emit_a1_tile(HPC + h, g, ps[:, 0, :])   # k col
                    emit_a1_tile(h, g, ps[:, 1, :])         # q col
                    qraw = tmp_pool.tile([128, 2, qn], FP16, tag="qraw",
                                         bufs=1, name=f"{R}qraw_{g}_{h}")
                    nc.scalar.activation(
                        qraw[:, :, :], ps[:, :, :],
                        mybir.ActivationFunctionType.Copy,
                        scale=float(1.0 / WS))
                    emit_rope(HPC + h, g, qraw[:, 0, :])
                    emit_rope(h, g, qraw[:, 1, :])

                # A1 group 0: pair h0's matmuls ran in the A2 scope;
                # rope it here, then the remaining pairs
                emit_rope(HPC + 0, 0, qraw0[:, 0, :])
                emit_rope(0, 0, qraw0[:, 1, :])
                for h in range(1, HPC):
                    emit_a1_pair(0, h)

                for g in range(NQT):
                    np2 = diag_per_qt * (g + 1) // 2
                    U = HPC * np2
                    s0 = slot[0]
                    # pre-queue side work for this stretch of units
                    if g >= 1:
                        for ct in range(NCT):
                            side_q.append(
                                [s0 + 2 + ct // 4,
                                 (lambda pqt, pct, pdve:
                                  lambda: emit_proj_ct(pqt, pct, pdve))(
                                     g - 1, ct, ct % 2 == 0)])
                    side_q.sort(key=lambda it: it[0])
                    if g >= 1:
                        for h in range(HPC):
                            emit_a1_pair(g, h)
                    for h in range(HPC):
                        for j in range(np2):
                            if j == 0 and h == 0:
                                y8_of_qt[g] = (
                                    y8_pool.tile([128, HPC, qn], FP8,
                                                 tag="y8h", name=f"{R}y8h_{g}"),
                                    y8_pool.tile([128, HPC, qn], FP8,
                                                 tag="y8l", name=f"{R}y8l_{g}"),
                                )
                            if j == 0:
                                state[(g, h)] = {
                                    "acc": acc_pool.tile(
                                        [128, 2, qn], FP16, tag="acc",
                                        name=f"{R}acc_{h}_{g}"),
                                    "psY": psY_pool.tile(
                                        [128, qn], FP32, tag="psY",
                                        name=f"{R}psY_{h}_{g}"),
                                }
                            ps = psS_pool.tile([128, 2, qn], FP32, tag="psS",
                                               name=f"{R}psS_{h}_{g}_{j}")
                            emit_scores(g, h, j, ps)
                            flush_prev(slot[0])
                            prev[0] = ps
                            prev[1] = (g, h, j, np2)
                            while side_q and side_q[0][0] <= slot[0]:
                                side_q.pop(0)[1]()
                            slot[0] += 1
                    # group end: ensure last unit's consumers are emitted
                    # before the next allocations recycle its psS slot
                    flush_prev(slot[0])
                    while side_q:
                        side_q.pop(0)[1]()
                # final projection for the last q-tile; alternate PSUM tags
                # (psS slots are idle by now) for a deeper evac pipeline
                for ct in range(NCT):
                    emit_proj_ct(NQT - 1, ct, ct % 2 == 0, tag_rot=True)
    nc.compile()
    return nc


_CACHE = {}


def _rope_tables_np(t_len, hd):
    inv_freq = 1.0 / (10000.0 ** (np.arange(0, hd, 2, dtype=np.float32) / hd))
    t = np.arange(t_len, dtype=np.float32)
    freqs = np.outer(t, inv_freq)
    emb = np.concatenate([freqs, freqs], axis=-1)
    return np.cos(emb)[:, ::2].astype(np.float32), np.sin(emb)[:, ::2].astype(np.float32)


def _static_arrays():
    if "static" not in _CACHE:
        cos_, sin_ = _rope_tables_np(T, HD)
        cosT = np.ascontiguousarray(cos_.T)   # (64, T)
        sinT = np.ascontiguousarray(sin_.T)
        # sin table halves are indexed by the *source* partition of the
        # rotate-half read: rows 64..127 hold -sin (multiplies x2 into the
        # low half), rows 0..63 hold +sin (multiplies x1 into the high half).
        cos2 = np.concatenate([cosT, cosT], axis=0).astype(np.float16)
        sin2 = np.concatenate([sinT, -sinT], axis=0).astype(np.float16)
        perm = np.concatenate([np.arange(0, HD, 2), np.arange(1, HD, 2)])
        p = np.arange(128)[:, None]
        f = np.arange(QN)[None, :]
        masks = np.concatenate(
            [(p <= (f - 128 * j)).astype(np.float16) for j in range(QN // 128)],
            axis=1)
        _CACHE["static"] = (cos2, sin2, perm, masks)
    return _CACHE["static"]


def _q8(a, np8):
    hi = a.astype(np8)
    lo = (a - hi.astype(np.float32)).astype(np8)
    return hi, lo


def _host_prep(x, w_qkv, w_proj):
    cos2, sin2, perm, masks = _static_arrays()
    np8 = mybir.dt.np(FP8)

    wq = w_qkv[:, 0 * C:1 * C]
    wk = w_qkv[:, 1 * C:2 * C]
    wv = w_qkv[:, 2 * C:3 * C]

    in_maps = []
    xq = {}
    for b in range(B):
        xT = np.ascontiguousarray(x[b].T)
        xq[b] = _q8(xT, np8)
    for c in range(N_CORES):
        b = c // GROUPS
        hg = c % GROUPS
        hs = slice(hg * HPC * HD, (hg + 1) * HPC * HD)
        wq_c = wq[:, hs].reshape(C, HPC, HD)[:, :, perm].reshape(C, HPC * HD)
        wk_c = wk[:, hs].reshape(C, HPC, HD)[:, :, perm].reshape(C, HPC * HD)
        wqk_c = np.concatenate([wq_c, wk_c], axis=1) * WS
        wqkh, wqkl = _q8(wqk_c, np8)
        wvh, wvl = _q8(wv[:, hs] * WS, np8)
        # wp arranged [d, s, i, c]: row d holds heads (2s+i) of this group
        wp_c = (w_proj[hs, :] * WS).reshape(2, 2, 128, C).transpose(2, 0, 1, 3)
        wph, wpl = _q8(np.ascontiguousarray(wp_c.reshape(128, 4 * C)), np8)
        x8h, x8l = xq[b]
        in_maps.append({
            "x8h": x8h, "x8l": x8l,
            "wqkh": wqkh, "wqkl": wqkl,
            "wvh": wvh, "wvl": wvl,
            "wph": wph, "wpl": wpl,
            "cos2": cos2, "sin2": sin2,
            "masks": masks,
        })
    return in_maps


class _PjrtRunner:
    """Caches the jitted shard_map callable so repeat kernel() calls skip
    retracing. Mirrors concourse.bass2jax.run_bass_via_pjrt."""

    def __init__(self, nc):
        import jax
        from jax.sharding import Mesh, PartitionSpec, NamedSharding
        from jax.experimental.shard_map import shard_map
        from concourse.bass2jax import (
            _bass_exec_p, install_neuronx_cc_hook, partition_id_tensor)

        install_neuronx_cc_hook()
        self.jax = jax
        partition_name = nc.partition_id_tensor.name if nc.partition_id_tensor else None
        in_names, out_names, out_avals = [], [], []
        for alloc in nc.m.functions[0].allocations:
            if not isinstance(alloc, mybir.MemoryLocationSet):
                continue
            name = alloc.memorylocations[0].name
            if alloc.kind == "ExternalInput":
                if name != partition_name:
                    in_names.append(name)
            elif alloc.kind == "ExternalOutput":
                out_names.append(name)
                out_avals.append(jax.core.ShapedArray(
                    tuple(alloc.tensor_shape), mybir.dt.np(alloc.dtype)))
        self.in_names, self.out_names, self.out_avals = in_names, out_names, out_avals
        n_params = len(in_names)
        n_outs = len(out_avals)
        bind_names = tuple(in_names + out_names +
                           ([partition_name] if partition_name else []))
        donate = tuple(range(n_params, n_params + n_outs))

        def _body(*args):
            operands = list(args)
            if partition_name:
                operands.append(partition_id_tensor())
            outs = _bass_exec_p.bind(
                *operands,
                out_avals=tuple(out_avals),
                in_names=bind_names,
                out_names=tuple(out_names),
                lowering_input_output_aliases=(),
                sim_require_finite=True,
                sim_require_nnan=True,
                nc=nc,
            )
            return tuple(outs)

        devices = jax.devices()[:N_CORES]
        self.mesh = Mesh(np.asarray(devices), ("core",))
        self.sharding = NamedSharding(self.mesh, PartitionSpec("core"))
        in_specs = (PartitionSpec("core"),) * (n_params + n_outs)
        out_specs = (PartitionSpec("core"),) * len(out_names)
        self.fn = jax.jit(
            shard_map(_body, mesh=self.mesh, in_specs=in_specs,
                      out_specs=out_specs, check_rep=False),
            donate_argnums=donate,
        )

    def run(self, in_maps):
        jax = self.jax
        concat = [
            np.concatenate([np.asarray(m[name]) for m in in_maps], axis=0)
            for name in self.in_names
        ]
        dev = [jax.device_put(c, self.sharding) for c in concat]
        zeros = [
            jax.device_put(
                np.zeros((N_CORES * a.shape[0], *a.shape[1:]), a.dtype),
                self.sharding)
            for a in self.out_avals
        ]
        outs = self.fn(*dev, *zeros)
        jax.block_until_ready(outs)
        res = []
        for c in range(N_CORES):
            d = {}
            for i, name in enumerate(self.out_names):
                a = np.asarray(outs[i])
                d[name] = a.reshape(N_CORES, *self.out_avals[i].shape)[c]
            res.append(d)
        return res


def _get_rt():
    if "rt" not in _CACHE:
        nc = _build_nc(T=T, C=C, HPC=HPC, n_cores=N_CORES, qn=QN, reps=1, an=AN)
        _CACHE["nc"] = nc
        _CACHE["rt"] = _PjrtRunner(nc) if axon_active() else None
    return _CACHE.get("nc"), _CACHE.get("rt")


def kernel(x, w_qkv, w_proj, n_head):
    assert int(n_head) == NH
    x = np.asarray(x, dtype=np.float32)
    w_qkv = np.asarray(w_qkv, dtype=np.float32)
    w_proj = np.asarray(w_proj, dtype=np.float32)
    assert x.shape == (B, T, C) and w_qkv.shape == (C, 3 * C) and w_proj.shape == (C, C)

    nc, rt = _get_rt()
    in_maps = _host_prep(x, w_qkv, w_proj)
    if rt is not None:
        results = rt.run(in_maps)
    else:
        results = run_bass_kernel_spmd(nc, in_maps,
                                       core_ids=list(range(N_CORES))).results

    out = np.zeros((B, T, C), dtype=np.float32)
    for c in range(N_CORES):
        b = c // GROUPS
        out[b] += results[c]["outT"].astype(np.float32).T
    return out
